# revision 12
# baseline (speedup 1.0000x reference)
"""Trainium2 Bass kernel for nn_BACKFLOW (batched backflow determinant).

Math (faithful to the reference):
    cols = first 32 column indices of nonzeros of (x == 1), row-major scan
    h    = tanh(x @ W1 + b1)                       [B, 4]
    h    = tanh(h @ W2 + b2)                       [B, 4]
    S    = tanh(einsum('bf,foe->boe', h, W3) + b3)[:, cols, :]   [B, 32, 32]
    out  = det(S)                                  [B]

Distribution: pure data parallel over the walker (batch) axis across 8
NeuronCores; the tiny MLP params and the selected W3/b3 slices (via `cols`)
are replicated to every core.

Device algorithm per core (4096 walkers = 32 tiles of 128; chunks [2,15,15]):
  * PE: x-tile transposes + W1/W2 matmuls (tanh on ScalarE), then per tile
    S = tanh(h2^T @ C) into SBUF as [128 walkers x tiles x 1024].
  * Batched unblocked LU on VectorE (broadcast APs, fp32), with:
      - adjacent-row pivoting only for k < 8 (swapped row negated so the
        det sign is preserved); smooth reciprocal guard r = piv/(piv^2+c^2).
      - the dominant rank-1 SUBTRACT offloaded for most tiles to a
        PE+ScalarE lane: ScalarE copies the trailing block SBUF->PSUM,
        PE accumulates -I @ tmp onto it (exact fp32), ScalarE copies back.
        VectorE keeps all the outer-product multiplies, a 1-2 row "strip"
        sub per tile (so the next step's pivot row is ready early), and
        the subs for the remaining tiles.
      - the two big chunks' LU loops are interleaved step-by-step so one
        chunk's PE/ScalarE lane hides under the other chunk's VectorE phase.
  * det = tree-product of the final diagonal; one transpose + DMA out.
fp32 everywhere: 16-bit LU storage and fp32r matmuls both fail the accuracy
budget (update-term rounding is amplified by elimination growth).
"""

import sys

if "/opt/trn_rl_repo" not in sys.path:
    sys.path.insert(0, "/opt/trn_rl_repo")

import numpy as np

NCORES = 8
B = 32768
O = 128          # orbitals
E = 32           # electrons == slater matrix size
H = 4            # MLP hidden
BC = B // NCORES     # walkers per core
PIV_CLAMP = 1e-6
NEIGH_UNTIL = 8      # adjacent-row pivoting for k < 8
CHUNKS = [2, 15, 15]
MLP_BLK = 5          # tiles per MLP block
WV = 3               # tiles per LU wave (tmp buffer granularity)
RAMP1 = 8            # chunk-1 head start over chunk-2 (steps)

_CACHE = {}


def _npe_for(ci, k, nt):
    """How many trailing tiles of chunk ci get the PE+ScalarE sub lane."""
    if ci == 0:
        return 0
    n = E - 1 - k
    npe = 10 if n >= 8 else 0
    return min(npe, nt)


def _patch_tile_tail_drain():
    """The tail drain TileContext emits carries >1 sem wait; this walrus
    build only accepts one sync wait per TPB_CTRL drain.  Split them."""
    import concourse.mybir as mybir
    import concourse.tile as tile_mod
    from concourse.tile import TileContext

    if getattr(TileContext, "_drain_patched", False):
        return
    _ScopedClock = tile_mod.ScopedClock

    def _patched(self, tick_clock, wait_clock):
        drain_inst = self.nc.sync.drain()
        wait_clock.add_sem_waits(
            drain_inst.ins, _ScopedClock({None: tick_clock.global_clock})
        )
        si = drain_inst.ins.sync_info
        if si is not None and len(si.on_wait) > 1:
            waits = list(si.on_wait)
            drain_inst.ins.sync_info = mybir.SyncInfo(
                on_wait=waits[:1], on_update=list(si.on_update)
            )
            for i in range(1, len(waits)):
                d2 = self.nc.sync.drain()
                d2.ins.sync_info = mybir.SyncInfo(on_wait=[waits[i]], on_update=[])
        self.nc.all_engine_barrier()
        assert self.sems is not None
        popped = self.nc._tile_sem_poison_stack.pop()
        assert popped is self._sem_poison
        self.nc.clear_and_free_semaphores(list(self.sems.allocated().values()))
        self.nc.all_engine_barrier()

    TileContext._drain_and_barrier = _patched
    TileContext._drain_patched = True


def _split_multi_waits(nc):
    """This walrus build accepts at most one sync-wait command per TPB
    instruction.  Move surplus waits onto same-engine NOPs inserted right
    before the owning instruction."""
    import concourse.mybir as mybir

    count = 0
    for blk in nc.m.functions[0].blocks:
        insts = list(blk.instructions)
        out = []
        changed = False
        for inst in insts:
            si = inst.sync_info
            if si is not None and len(si.on_wait) > 1:
                waits = list(si.on_wait)
                for w in waits[:-1]:
                    count += 1
                    nop = mybir.InstNoOp(
                        name=f"Wsplit-{count}", engine=inst.engine
                    )
                    nop.sync_info = mybir.SyncInfo(on_wait=[w], on_update=[])
                    out.append(nop)
                inst.sync_info = mybir.SyncInfo(
                    on_wait=[waits[-1]], on_update=list(si.on_update)
                )
                changed = True
            out.append(inst)
        if changed:
            blk.instructions = out
    return count


def _build_bass(include_bias):
    import concourse.bass as bass
    import concourse.mybir as mybir
    from concourse.masks import make_identity
    from concourse.tile import TileContext

    _patch_tile_tail_drain()

    f32 = mybir.dt.float32
    u32 = mybir.dt.uint32
    Alu = mybir.AluOpType
    Act = mybir.ActivationFunctionType

    nc = bass.Bass()
    xc = nc.dram_tensor("xc", [BC, O], f32, kind="ExternalInput")
    w1 = nc.dram_tensor("w1", [O, H], f32, kind="ExternalInput")
    w2 = nc.dram_tensor("w2", [H, H], f32, kind="ExternalInput")
    bias1 = nc.dram_tensor("bias1", [H, 1], f32, kind="ExternalInput")
    bias2 = nc.dram_tensor("bias2", [H, 1], f32, kind="ExternalInput")
    caug = nc.dram_tensor("caug", [H + 1, E * E], f32, kind="ExternalInput")
    out = nc.dram_tensor("out", [BC // 128, 128], f32, kind="ExternalOutput")

    NTX = max(CHUNKS)
    CC2 = PIV_CLAMP * PIV_CLAMP
    MMF = 512  # PSUM accumulation-group limit per matmul

    with TileContext(nc) as tc:
        with (
            tc.tile_pool(name="consts", bufs=1) as consts,
            tc.tile_pool(name="mlp", bufs=2) as mlp,
            tc.tile_pool(name="apool", bufs=1) as apool,
            tc.tile_pool(name="work", bufs=1) as work,
            tc.tile_pool(name="wave", bufs=3) as wave,
            tc.tile_pool(name="strip", bufs=2) as strip,
            tc.tile_pool(name="lane", bufs=2, space="PSUM") as lane,
            tc.tile_pool(name="ps_m", bufs=1, space="PSUM") as ps_m,
            tc.tile_pool(name="ps_t", bufs=1, space="PSUM") as ps_t,
            tc.tile_pool(name="ps_h", bufs=1, space="PSUM") as ps_h,
        ):
            ident = consts.tile([128, 128], f32)
            make_identity(nc, ident)
            identn = consts.tile([128, 128], f32)
            nc.vector.tensor_scalar_mul(identn, ident, -1.0)
            # 1-row zero stationary: "+0" fence matmuls register the PSUM
            # accumulate as a tracked write (the start=False MM alone is
            # invisible to the dep tracker -> copy-back raced it)
            zrow = consts.tile([1, 128], f32)
            nc.vector.memset(zrow, 0.0)
            zmov = consts.tile([1, 1], f32)
            nc.vector.memset(zmov, 0.0)
            w1t = consts.tile([O, H], f32)
            nc.sync.dma_start(w1t, w1[:, :])
            w2t = consts.tile([H, H], f32)
            nc.sync.dma_start(w2t, w2[:, :])
            b1t = consts.tile([H, 1], f32)
            nc.sync.dma_start(b1t, bias1[:, :])
            b2t = consts.tile([H, 1], f32)
            nc.sync.dma_start(b2t, bias2[:, :])
            cgt = consts.tile([H, E * E], f32)
            nc.sync.dma_start(cgt, caug[0:H, :])
            if include_bias:
                b3r = consts.tile([1, E * E], f32)
                nc.sync.dma_start(b3r, caug[H : H + 1, :])
                onesr = consts.tile([1, 128], f32)
                nc.vector.memset(onesr, 1.0)

            detall = consts.tile([128, BC // 128], f32)
            guard_s = [consts.tile([128, 1], f32, tag=f"gs{c}", name=f"gs{c}")
                       for c in range(len(CHUNKS))]
            guard_d = [consts.tile([128, 1], f32, tag=f"gd{c}", name=f"gd{c}")
                       for c in range(len(CHUNKS))]
            had_lane = [False] * len(CHUNKS)

            # per-chunk S tiles
            A_c = [apool.tile([128, nt, E * E], f32, tag=f"A{ci}", name=f"A{ci}")
                   for ci, nt in enumerate(CHUNKS)]
            A4_c = [A.rearrange("p t (i j) -> p t i j", i=E) for A in A_c]
            offs = [sum(CHUNKS[:ci]) for ci in range(len(CHUNKS))]

            # persistent LU scratch
            rcp = work.tile([128, NTX], f32)
            pv2 = work.tile([128, NTX], f32)
            nsq = work.tile([128, NTX, 2], f32)
            maskU = work.tile([128, NTX], u32)
            rowp = work.tile([128, NTX, E], f32)
            trow = work.tile([128, NTX, E], f32)

            def emit_mlp_chunk(ci):
                for b0 in range(0, CHUNKS[ci], MLP_BLK):
                    emit_mlp_block(ci, b0)

            def emit_mlp_block(ci, b0):
                nt = CHUNKS[ci]
                toff = offs[ci]
                A = A_c[ci]
                if True:
                    bt = min(MLP_BLK, nt - b0)
                    bw = bt * 128
                    w0 = (toff + b0) * 128
                    xx = mlp.tile([128, MLP_BLK, O], f32, tag="xx")
                    nc.sync.dma_start(
                        xx[:, :bt],
                        xc[w0 : w0 + bw, :].rearrange("(t p) o -> p t o", p=128),
                    )
                    xT = mlp.tile([O, MLP_BLK, 128], f32, tag="xT")
                    for t in range(bt):
                        pst = ps_t.tile([128, 128], f32, tag="pst")
                        nc.tensor.transpose(pst, xx[:, t, :], ident)
                        nc.scalar.copy(xT[:, t, :], pst)

                    xTf = xT.rearrange("p t w -> p (t w)")
                    h1 = mlp.tile([H, MLP_BLK * 128], f32, tag="h1")
                    for s0 in range(0, bw, 512):
                        sl = min(512, bw - s0)
                        ph = ps_h.tile([H, 512], f32, tag="ph")
                        nc.tensor.matmul(ph[:, :sl], w1t, xTf[:, s0 : s0 + sl])
                        nc.scalar.activation(
                            h1[:, s0 : s0 + sl], ph[:, :sl], Act.Tanh, bias=b1t
                        )
                    h2a = mlp.tile([H, MLP_BLK * 128], f32, tag="h2a")
                    for s0 in range(0, bw, 512):
                        sl = min(512, bw - s0)
                        ph2 = ps_h.tile([H, 512], f32, tag="ph")
                        nc.tensor.matmul(ph2[:, :sl], w2t, h1[:, s0 : s0 + sl])
                        nc.scalar.activation(
                            h2a[0:H, s0 : s0 + sl], ph2[:, :sl], Act.Tanh, bias=b2t
                        )
                    for t in range(bt):
                        pm = ps_m.tile([128, E * E], f32, tag="pm")
                        for s in range(2):
                            nc.tensor.matmul(
                                pm[:, s * 512 : (s + 1) * 512],
                                h2a[:, t * 128 : (t + 1) * 128],
                                cgt[:, s * 512 : (s + 1) * 512],
                                start=True,
                                stop=not include_bias,
                            )
                            if include_bias:
                                nc.tensor.matmul(
                                    pm[:, s * 512 : (s + 1) * 512],
                                    onesr,
                                    b3r[:, s * 512 : (s + 1) * 512],
                                    start=False,
                                    stop=True,
                                )
                        nc.scalar.activation(A[:, b0 + t, :], pm, Act.Tanh)

            def emit_phase(ci, k):
                """One LU step k for chunk ci."""
                nt = CHUNKS[ci]
                A = A_c[ci]
                A4 = A4_c[ci]
                if k < NEIGH_UNTIL and k < E - 1:
                    L = E - k
                    pcand = A[:, :, k * 33 : k * 33 + 33 : 32]
                    nc.vector.tensor_mul(nsq[:, :nt], pcand, pcand)
                    nc.vector.tensor_tensor(
                        maskU[:, :nt], nsq[:, :nt, 1], nsq[:, :nt, 0], Alu.is_gt
                    )
                    mb = maskU[:, :nt, None].broadcast_to([128, nt, L])
                    rK = A4[:, :, k, k:]
                    rK1 = A4[:, :, k + 1, k:]
                    nc.vector.tensor_scalar_mul(trow[:, :nt, :L], rK, -1.0)
                    nc.vector.copy_predicated(rK, mb, rK1)
                    nc.vector.copy_predicated(rK1, mb, trow[:, :nt, :L])

                if k >= E - 1:
                    return
                n = E - 1 - k
                piv = A4[:, :, k, k]
                # r = piv / (piv^2 + c^2): smooth sign-correct guarded recip
                nc.vector.tensor_mul(pv2[:, :nt], piv, piv)
                nc.vector.tensor_scalar(
                    pv2[:, :nt], pv2[:, :nt], CC2, None, Alu.add
                )
                nc.vector.reciprocal(pv2[:, :nt], pv2[:, :nt])
                nc.vector.tensor_mul(rcp[:, :nt], piv, pv2[:, :nt])
                row = A4[:, :, k, k + 1 :]
                nc.vector.tensor_mul(
                    rowp[:, :nt, :n],
                    row,
                    rcp[:, :nt, None].broadcast_to([128, nt, n]),
                )

                npe = _npe_for(ci, k, nt)
                srows = 2 if (k + 1) < NEIGH_UNTIL else 1
                if n <= srows:
                    npe = 0
                ndve = nt - npe
                if had_lane[ci]:
                    # gate this chunk's DVE work on its previous phase's
                    # ScalarE copy-backs (the tracker misses some strided
                    # cb->DVE RAW edges; ScE is in-order so the marker op
                    # emitted after those cbs covers them all)
                    nc.vector.tensor_copy(guard_d[ci], guard_s[ci])
                    had_lane[ci] = False

                if npe:
                    # strip rows k+1..k+srows of the PE tiles on DVE so the
                    # next step's pivot row / chain is ready early
                    tws = strip.tile([128, NTX, 2, E], f32, tag="ts")
                    scol = A4[:, ndve:, k + 1 : k + 1 + srows, k]
                    nc.vector.tensor_mul(
                        tws[:, :npe, :srows, :n],
                        scol[:, :, :, None].broadcast_to([128, npe, srows, n]),
                        rowp[:, ndve:, None, :n].broadcast_to(
                            [128, npe, srows, n]
                        ),
                    )
                    nc.vector.tensor_sub(
                        A4[:, ndve:, k + 1 : k + 1 + srows, k + 1 :],
                        A4[:, ndve:, k + 1 : k + 1 + srows, k + 1 :],
                        tws[:, :npe, :srows, :n],
                    )
                # PE-lane tiles first so the lane starts early
                nr = n - srows
                for g0 in range(ndve, nt, WV):
                    g1 = min(g0 + WV, nt)
                    wn = g1 - g0
                    tw = wave.tile([128, WV, nr, n], f32, tag="tw")
                    col = A4[:, g0:g1, k + 1 + srows :, k]
                    nc.vector.tensor_mul(
                        tw[:, :wn],
                        col[:, :, :, None].broadcast_to([128, wn, nr, n]),
                        rowp[:, g0:g1, None, :n].broadcast_to([128, wn, nr, n]),
                    )
                    twf = tw.rearrange("p t i j -> p t (i j)")
                    region = nr * n
                    for t in range(g0, g1):
                        lt = lane.tile([128, 1024], f32, tag="lt")
                        nc.scalar.copy(
                            lt[:, :region].rearrange("p (i j) -> p i j", i=nr),
                            A4[:, t, k + 1 + srows :, k + 1 :],
                        )
                        for s0 in range(0, region, MMF):
                            sl = min(MMF, region - s0)
                            nc.tensor.matmul(
                                lt[:, s0 : s0 + sl],
                                identn,
                                twf[:, t - g0, s0 : s0 + sl],
                                start=False,
                                stop=True,
                                skip_group_check=True,
                            )
                        nc.tensor.matmul(
                            lt[:, 0:1], zrow, zmov,
                            start=False, stop=True, skip_group_check=True,
                        )
                        nc.scalar.copy(
                            A4[:, t, k + 1 + srows :, k + 1 :],
                            lt[:, :region].rearrange("p (i j) -> p i j", i=nr),
                        )
                if npe:
                    nc.scalar.copy(guard_s[ci], identn[:, 0:1])
                    had_lane[ci] = True
                # DVE tiles
                for g0 in range(0, ndve, WV):
                    g1 = min(g0 + WV, ndve)
                    wn = g1 - g0
                    tw = wave.tile([128, WV, n, n], f32, tag="tw")
                    col = A4[:, g0:g1, k + 1 :, k]
                    nc.vector.tensor_mul(
                        tw[:, :wn],
                        col[:, :, :, None].broadcast_to([128, wn, n, n]),
                        rowp[:, g0:g1, None, :n].broadcast_to([128, wn, n, n]),
                    )
                    nc.vector.tensor_sub(
                        A4[:, g0:g1, k + 1 :, k + 1 :],
                        A4[:, g0:g1, k + 1 :, k + 1 :],
                        tw[:, : g1 - g0],
                    )

            def emit_det(ci):
                nt = CHUNKS[ci]
                A = A_c[ci]
                toff = offs[ci]
                diag = A[:, :, ::33]
                nc.vector.tensor_mul(
                    rowp[:, :nt, :16], diag[:, :, :16], diag[:, :, 16:]
                )
                nc.vector.tensor_mul(
                    rowp[:, :nt, :8], rowp[:, :nt, :8], rowp[:, :nt, 8:16]
                )
                nc.vector.tensor_mul(
                    rowp[:, :nt, :4], rowp[:, :nt, :4], rowp[:, :nt, 4:8]
                )
                nc.vector.tensor_mul(
                    rowp[:, :nt, :2], rowp[:, :nt, :2], rowp[:, :nt, 2:4]
                )
                nc.vector.tensor_mul(
                    detall[:, toff : toff + nt],
                    rowp[:, :nt, 0],
                    rowp[:, :nt, 1],
                )

            # ---- schedule ----
            emit_mlp_chunk(0)
            emit_mlp_chunk(1)
            for k in range(E):
                emit_phase(0, k)
            emit_det(0)
            # chunk-1 ramp; chunk-2 MLP blocks slot between the phases
            c2blocks = list(range(0, CHUNKS[2], MLP_BLK))
            for k in range(RAMP1):
                emit_phase(1, k)
                if k < len(c2blocks):
                    emit_mlp_block(2, c2blocks[k])
            # interleave chunk 1 (ahead by RAMP1) with chunk 2
            for k in range(E):
                if k + RAMP1 < E:
                    emit_phase(1, k + RAMP1)
                emit_phase(2, k)
            emit_det(1)
            emit_det(2)

            # ---- emit dets: [128, 32] -> [32, 128] -> DRAM ----
            psd = ps_t.tile([BC // 128, 128], f32, tag="pst")
            nc.tensor.transpose(psd, detall, ident)
            dsb = consts.tile([BC // 128, 128], f32)
            nc.scalar.copy(dsb, psd)
            nc.sync.dma_start(out[:, :], dsb)

    nsplit = _split_multi_waits(nc)
    if nsplit:
        print(f"[kernel] split {nsplit} surplus sync waits onto NOPs")
    return nc


def _get_nc(include_bias=False):
    key = ("nc", bool(include_bias))
    if key not in _CACHE:
        _CACHE[key] = _build_bass(include_bias)
    return _CACHE[key]


def _first_nonzero_cols(x: np.ndarray) -> np.ndarray:
    """First E column indices of nonzeros of (x == 1) in row-major order."""
    cols = []
    for r in range(x.shape[0]):
        nz = np.flatnonzero(x[r] == 1)
        take = min(E - len(cols), nz.size)
        if take:
            cols.extend(nz[:take].tolist())
        if len(cols) >= E:
            break
    cols = cols[:E] + [0] * (E - len(cols))  # jnp.nonzero(size=E) zero-fill
    return np.asarray(cols, dtype=np.int64)


def kernel(x, W1, b1, W2, b2, W3, b3):
    from concourse import bass_utils

    x = np.ascontiguousarray(np.asarray(x, dtype=np.float32))
    W1 = np.asarray(W1, dtype=np.float32)
    b1 = np.asarray(b1, dtype=np.float32)
    W2 = np.asarray(W2, dtype=np.float32)
    b2 = np.asarray(b2, dtype=np.float32)
    W3 = np.asarray(W3, dtype=np.float32)
    b3 = np.asarray(b3, dtype=np.float32)

    cols = _first_nonzero_cols(x)
    csel = W3[:, cols, :].reshape(H, E * E)
    bsel = b3[cols, :].reshape(1, E * E)
    caug = np.ascontiguousarray(np.concatenate([csel, bsel], axis=0))

    shared = {
        "w1": W1,
        "w2": W2,
        "bias1": b1.reshape(H, 1),
        "bias2": b2.reshape(H, 1),
        "caug": caug,
    }
    in_maps = [
        {"xc": x[c * BC : (c + 1) * BC], **shared} for c in range(NCORES)
    ]

    nc = _get_nc(include_bias=bool(np.any(bsel)))
    res = bass_utils.run_bass_kernel_spmd(nc, in_maps, core_ids=list(range(NCORES)))
    det = np.concatenate(
        [np.asarray(res.results[c]["out"]).reshape(BC) for c in range(NCORES)]
    )
    return det.astype(np.float32)


# revision 13
# speedup vs baseline: 1.0740x; 1.0740x over previous
"""Trainium2 Bass kernel for nn_BACKFLOW (batched backflow determinant).

Math (faithful to the reference):
    cols = first 32 column indices of nonzeros of (x == 1), row-major scan
    h    = tanh(x @ W1 + b1)                       [B, 4]
    h    = tanh(h @ W2 + b2)                       [B, 4]
    S    = tanh(einsum('bf,foe->boe', h, W3) + b3)[:, cols, :]   [B, 32, 32]
    out  = det(S)                                  [B]

Distribution: pure data parallel over the walker (batch) axis across 8
NeuronCores; the tiny MLP params and the selected W3/b3 slices (via `cols`)
are replicated to every core.

Device algorithm per core (4096 walkers = 32 tiles of 128; chunks [2,15,15]):
  * PE: x-tile transposes + W1/W2 matmuls (tanh on ScalarE), then per tile
    S = tanh(h2^T @ C) into SBUF as [128 walkers x tiles x 1024].
  * Batched unblocked LU on VectorE (broadcast APs, fp32), with:
      - adjacent-row pivoting only for k < 8 (swapped row negated so the
        det sign is preserved); smooth reciprocal guard r = piv/(piv^2+c^2).
      - the dominant rank-1 SUBTRACT offloaded for most tiles to a
        PE+ScalarE lane: ScalarE copies the trailing block SBUF->PSUM,
        PE accumulates -I @ tmp onto it (exact fp32), ScalarE copies back.
        VectorE keeps all the outer-product multiplies, a 1-2 row "strip"
        sub per tile (so the next step's pivot row is ready early), and
        the subs for the remaining tiles.
      - the two big chunks' LU loops are interleaved step-by-step so one
        chunk's PE/ScalarE lane hides under the other chunk's VectorE phase.
  * det = tree-product of the final diagonal; one transpose + DMA out.
fp32 everywhere: 16-bit LU storage and fp32r matmuls both fail the accuracy
budget (update-term rounding is amplified by elimination growth).
"""

import sys

if "/opt/trn_rl_repo" not in sys.path:
    sys.path.insert(0, "/opt/trn_rl_repo")

import numpy as np

NCORES = 8
B = 32768
O = 128          # orbitals
E = 32           # electrons == slater matrix size
H = 4            # MLP hidden
BC = B // NCORES     # walkers per core
PIV_CLAMP = 1e-6
NEIGH_UNTIL = 8      # adjacent-row pivoting for k < 8
CHUNKS = [2, 15, 15]
MLP_BLK = 5          # tiles per MLP block
WV = 3               # tiles per LU wave (tmp buffer granularity)
RAMP1 = 8            # chunk-1 head start over chunk-2 (steps)

_CACHE = {}


def _npe_for(ci, k, nt):
    """How many trailing tiles of chunk ci get the PE+ScalarE sub lane."""
    if ci == 0:
        return 0
    n = E - 1 - k
    if n >= 16:
        npe = 10
    elif n >= 12:
        npe = 8
    elif n >= 8:
        npe = 5
    else:
        npe = 0
    return min(npe, nt)


def _patch_tile_tail_drain():
    """The tail drain TileContext emits carries >1 sem wait; this walrus
    build only accepts one sync wait per TPB_CTRL drain.  Split them."""
    import concourse.mybir as mybir
    import concourse.tile as tile_mod
    from concourse.tile import TileContext

    if getattr(TileContext, "_drain_patched", False):
        return
    _ScopedClock = tile_mod.ScopedClock

    def _patched(self, tick_clock, wait_clock):
        drain_inst = self.nc.sync.drain()
        wait_clock.add_sem_waits(
            drain_inst.ins, _ScopedClock({None: tick_clock.global_clock})
        )
        si = drain_inst.ins.sync_info
        if si is not None and len(si.on_wait) > 1:
            waits = list(si.on_wait)
            drain_inst.ins.sync_info = mybir.SyncInfo(
                on_wait=waits[:1], on_update=list(si.on_update)
            )
            for i in range(1, len(waits)):
                d2 = self.nc.sync.drain()
                d2.ins.sync_info = mybir.SyncInfo(on_wait=[waits[i]], on_update=[])
        self.nc.all_engine_barrier()
        assert self.sems is not None
        popped = self.nc._tile_sem_poison_stack.pop()
        assert popped is self._sem_poison
        self.nc.clear_and_free_semaphores(list(self.sems.allocated().values()))
        self.nc.all_engine_barrier()

    TileContext._drain_and_barrier = _patched
    TileContext._drain_patched = True


def _split_multi_waits(nc):
    """This walrus build accepts at most one sync-wait command per TPB
    instruction.  Move surplus waits onto same-engine NOPs inserted right
    before the owning instruction."""
    import concourse.mybir as mybir

    count = 0
    for blk in nc.m.functions[0].blocks:
        insts = list(blk.instructions)
        out = []
        changed = False
        for inst in insts:
            si = inst.sync_info
            if si is not None and len(si.on_wait) > 1:
                waits = list(si.on_wait)
                for w in waits[:-1]:
                    count += 1
                    nop = mybir.InstNoOp(
                        name=f"Wsplit-{count}", engine=inst.engine
                    )
                    nop.sync_info = mybir.SyncInfo(on_wait=[w], on_update=[])
                    out.append(nop)
                inst.sync_info = mybir.SyncInfo(
                    on_wait=[waits[-1]], on_update=list(si.on_update)
                )
                changed = True
            out.append(inst)
        if changed:
            blk.instructions = out
    return count


def _build_bass(include_bias):
    import concourse.bass as bass
    import concourse.mybir as mybir
    from concourse.masks import make_identity
    from concourse.tile import TileContext

    _patch_tile_tail_drain()

    f32 = mybir.dt.float32
    u32 = mybir.dt.uint32
    Alu = mybir.AluOpType
    Act = mybir.ActivationFunctionType

    nc = bass.Bass()
    xc = nc.dram_tensor("xc", [BC, O], f32, kind="ExternalInput")
    w1 = nc.dram_tensor("w1", [O, H], f32, kind="ExternalInput")
    w2 = nc.dram_tensor("w2", [H, H], f32, kind="ExternalInput")
    bias1 = nc.dram_tensor("bias1", [H, 1], f32, kind="ExternalInput")
    bias2 = nc.dram_tensor("bias2", [H, 1], f32, kind="ExternalInput")
    caug = nc.dram_tensor("caug", [H + 1, E * E], f32, kind="ExternalInput")
    out = nc.dram_tensor("out", [BC // 128, 128], f32, kind="ExternalOutput")

    NTX = max(CHUNKS)
    CC2 = PIV_CLAMP * PIV_CLAMP
    MMF = 512  # PSUM accumulation-group limit per matmul

    with TileContext(nc) as tc:
        with (
            tc.tile_pool(name="consts", bufs=1) as consts,
            tc.tile_pool(name="mlp", bufs=2) as mlp,
            tc.tile_pool(name="apool", bufs=1) as apool,
            tc.tile_pool(name="work", bufs=1) as work,
            tc.tile_pool(name="wave", bufs=3) as wave,
            tc.tile_pool(name="strip", bufs=2) as strip,
            tc.tile_pool(name="lane", bufs=2, space="PSUM") as lane,
            tc.tile_pool(name="ps_m", bufs=1, space="PSUM") as ps_m,
            tc.tile_pool(name="ps_t", bufs=1, space="PSUM") as ps_t,
            tc.tile_pool(name="ps_h", bufs=1, space="PSUM") as ps_h,
        ):
            ident = consts.tile([128, 128], f32)
            make_identity(nc, ident)
            identn = consts.tile([128, 128], f32)
            nc.vector.tensor_scalar_mul(identn, ident, -1.0)
            # 1-row zero stationary: "+0" fence matmuls register the PSUM
            # accumulate as a tracked write (the start=False MM alone is
            # invisible to the dep tracker -> copy-back raced it)
            zrow = consts.tile([1, 128], f32)
            nc.vector.memset(zrow, 0.0)
            zmov = consts.tile([1, 1], f32)
            nc.vector.memset(zmov, 0.0)
            w1t = consts.tile([O, H], f32)
            nc.sync.dma_start(w1t, w1[:, :])
            w2t = consts.tile([H, H], f32)
            nc.sync.dma_start(w2t, w2[:, :])
            b1t = consts.tile([H, 1], f32)
            nc.sync.dma_start(b1t, bias1[:, :])
            b2t = consts.tile([H, 1], f32)
            nc.sync.dma_start(b2t, bias2[:, :])
            cgt = consts.tile([H, E * E], f32)
            nc.sync.dma_start(cgt, caug[0:H, :])
            if include_bias:
                b3r = consts.tile([1, E * E], f32)
                nc.sync.dma_start(b3r, caug[H : H + 1, :])
                onesr = consts.tile([1, 128], f32)
                nc.vector.memset(onesr, 1.0)

            detall = consts.tile([128, BC // 128], f32)
            guard_s = [consts.tile([128, 1], f32, tag=f"gs{c}", name=f"gs{c}")
                       for c in range(len(CHUNKS))]
            guard_d = [consts.tile([128, 1], f32, tag=f"gd{c}", name=f"gd{c}")
                       for c in range(len(CHUNKS))]
            had_lane = [False] * len(CHUNKS)

            # per-chunk S tiles
            A_c = [apool.tile([128, nt, E * E], f32, tag=f"A{ci}", name=f"A{ci}")
                   for ci, nt in enumerate(CHUNKS)]
            A4_c = [A.rearrange("p t (i j) -> p t i j", i=E) for A in A_c]
            offs = [sum(CHUNKS[:ci]) for ci in range(len(CHUNKS))]

            # persistent LU scratch
            rcp = work.tile([128, NTX], f32)
            pv2 = work.tile([128, NTX], f32)
            nsq = work.tile([128, NTX, 2], f32)
            maskU = work.tile([128, NTX], u32)
            rowp = work.tile([128, NTX, E], f32)
            trow = work.tile([128, NTX, E], f32)

            def emit_mlp_chunk(ci):
                for b0 in range(0, CHUNKS[ci], MLP_BLK):
                    emit_mlp_block(ci, b0)

            def emit_mlp_block(ci, b0):
                nt = CHUNKS[ci]
                toff = offs[ci]
                A = A_c[ci]
                if True:
                    bt = min(MLP_BLK, nt - b0)
                    bw = bt * 128
                    w0 = (toff + b0) * 128
                    xx = mlp.tile([128, MLP_BLK, O], f32, tag="xx")
                    nc.sync.dma_start(
                        xx[:, :bt],
                        xc[w0 : w0 + bw, :].rearrange("(t p) o -> p t o", p=128),
                    )
                    xT = mlp.tile([O, MLP_BLK, 128], f32, tag="xT")
                    for t in range(bt):
                        pst = ps_t.tile([128, 128], f32, tag="pst")
                        nc.tensor.transpose(pst, xx[:, t, :], ident)
                        nc.scalar.copy(xT[:, t, :], pst)

                    xTf = xT.rearrange("p t w -> p (t w)")
                    h1 = mlp.tile([H, MLP_BLK * 128], f32, tag="h1")
                    for s0 in range(0, bw, 512):
                        sl = min(512, bw - s0)
                        ph = ps_h.tile([H, 512], f32, tag="ph")
                        nc.tensor.matmul(ph[:, :sl], w1t, xTf[:, s0 : s0 + sl])
                        nc.scalar.activation(
                            h1[:, s0 : s0 + sl], ph[:, :sl], Act.Tanh, bias=b1t
                        )
                    h2a = mlp.tile([H, MLP_BLK * 128], f32, tag="h2a")
                    for s0 in range(0, bw, 512):
                        sl = min(512, bw - s0)
                        ph2 = ps_h.tile([H, 512], f32, tag="ph")
                        nc.tensor.matmul(ph2[:, :sl], w2t, h1[:, s0 : s0 + sl])
                        nc.scalar.activation(
                            h2a[0:H, s0 : s0 + sl], ph2[:, :sl], Act.Tanh, bias=b2t
                        )
                    for t in range(bt):
                        pm = ps_m.tile([128, E * E], f32, tag="pm")
                        for s in range(2):
                            nc.tensor.matmul(
                                pm[:, s * 512 : (s + 1) * 512],
                                h2a[:, t * 128 : (t + 1) * 128],
                                cgt[:, s * 512 : (s + 1) * 512],
                                start=True,
                                stop=not include_bias,
                            )
                            if include_bias:
                                nc.tensor.matmul(
                                    pm[:, s * 512 : (s + 1) * 512],
                                    onesr,
                                    b3r[:, s * 512 : (s + 1) * 512],
                                    start=False,
                                    stop=True,
                                )
                        nc.scalar.activation(A[:, b0 + t, :], pm, Act.Tanh)

            def emit_phase(ci, k):
                """One LU step k for chunk ci."""
                nt = CHUNKS[ci]
                A = A_c[ci]
                A4 = A4_c[ci]
                if k < NEIGH_UNTIL and k < E - 1:
                    L = E - k
                    pcand = A[:, :, k * 33 : k * 33 + 33 : 32]
                    nc.vector.tensor_mul(nsq[:, :nt], pcand, pcand)
                    nc.vector.tensor_tensor(
                        maskU[:, :nt], nsq[:, :nt, 1], nsq[:, :nt, 0], Alu.is_gt
                    )
                    mb = maskU[:, :nt, None].broadcast_to([128, nt, L])
                    rK = A4[:, :, k, k:]
                    rK1 = A4[:, :, k + 1, k:]
                    nc.vector.tensor_scalar_mul(trow[:, :nt, :L], rK, -1.0)
                    nc.vector.copy_predicated(rK, mb, rK1)
                    nc.vector.copy_predicated(rK1, mb, trow[:, :nt, :L])

                if k >= E - 1:
                    return
                n = E - 1 - k
                piv = A4[:, :, k, k]
                # r = piv / (piv^2 + c^2): smooth sign-correct guarded recip
                nc.vector.tensor_mul(pv2[:, :nt], piv, piv)
                nc.vector.tensor_scalar(
                    pv2[:, :nt], pv2[:, :nt], CC2, None, Alu.add
                )
                nc.vector.reciprocal(pv2[:, :nt], pv2[:, :nt])
                nc.vector.tensor_mul(rcp[:, :nt], piv, pv2[:, :nt])
                row = A4[:, :, k, k + 1 :]
                nc.vector.tensor_mul(
                    rowp[:, :nt, :n],
                    row,
                    rcp[:, :nt, None].broadcast_to([128, nt, n]),
                )

                npe = _npe_for(ci, k, nt)
                srows = 2 if (k + 1) < NEIGH_UNTIL else 1
                if n <= srows:
                    npe = 0
                ndve = nt - npe
                if had_lane[ci]:
                    # gate this chunk's DVE work on its previous phase's
                    # ScalarE copy-backs (the tracker misses some strided
                    # cb->DVE RAW edges; ScE is in-order so the marker op
                    # emitted after those cbs covers them all)
                    nc.vector.tensor_copy(guard_d[ci], guard_s[ci])
                    had_lane[ci] = False

                if npe:
                    # strip rows k+1..k+srows of the PE tiles on DVE so the
                    # next step's pivot row / chain is ready early
                    tws = strip.tile([128, NTX, 2, E], f32, tag="ts")
                    scol = A4[:, ndve:, k + 1 : k + 1 + srows, k]
                    nc.vector.tensor_mul(
                        tws[:, :npe, :srows, :n],
                        scol[:, :, :, None].broadcast_to([128, npe, srows, n]),
                        rowp[:, ndve:, None, :n].broadcast_to(
                            [128, npe, srows, n]
                        ),
                    )
                    nc.vector.tensor_sub(
                        A4[:, ndve:, k + 1 : k + 1 + srows, k + 1 :],
                        A4[:, ndve:, k + 1 : k + 1 + srows, k + 1 :],
                        tws[:, :npe, :srows, :n],
                    )
                # PE-lane tiles first so the lane starts early
                nr = n - srows
                for g0 in range(ndve, nt, WV):
                    g1 = min(g0 + WV, nt)
                    wn = g1 - g0
                    tw = wave.tile([128, WV, nr, n], f32, tag="tw")
                    col = A4[:, g0:g1, k + 1 + srows :, k]
                    nc.vector.tensor_mul(
                        tw[:, :wn],
                        col[:, :, :, None].broadcast_to([128, wn, nr, n]),
                        rowp[:, g0:g1, None, :n].broadcast_to([128, wn, nr, n]),
                    )
                    twf = tw.rearrange("p t i j -> p t (i j)")
                    region = nr * n
                    for t in range(g0, g1):
                        lt = lane.tile([128, 1024], f32, tag="lt")
                        nc.scalar.copy(
                            lt[:, :region].rearrange("p (i j) -> p i j", i=nr),
                            A4[:, t, k + 1 + srows :, k + 1 :],
                        )
                        for s0 in range(0, region, MMF):
                            sl = min(MMF, region - s0)
                            nc.tensor.matmul(
                                lt[:, s0 : s0 + sl],
                                identn,
                                twf[:, t - g0, s0 : s0 + sl],
                                start=False,
                                stop=True,
                                skip_group_check=True,
                            )
                        nc.tensor.matmul(
                            lt[:, 0:1], zrow, zmov,
                            start=False, stop=True, skip_group_check=True,
                        )
                        nc.scalar.copy(
                            A4[:, t, k + 1 + srows :, k + 1 :],
                            lt[:, :region].rearrange("p (i j) -> p i j", i=nr),
                        )
                if npe:
                    nc.scalar.copy(guard_s[ci], identn[:, 0:1])
                    had_lane[ci] = True
                # DVE tiles
                for g0 in range(0, ndve, WV):
                    g1 = min(g0 + WV, ndve)
                    wn = g1 - g0
                    tw = wave.tile([128, WV, n, n], f32, tag="tw")
                    col = A4[:, g0:g1, k + 1 :, k]
                    nc.vector.tensor_mul(
                        tw[:, :wn],
                        col[:, :, :, None].broadcast_to([128, wn, n, n]),
                        rowp[:, g0:g1, None, :n].broadcast_to([128, wn, n, n]),
                    )
                    nc.vector.tensor_sub(
                        A4[:, g0:g1, k + 1 :, k + 1 :],
                        A4[:, g0:g1, k + 1 :, k + 1 :],
                        tw[:, : g1 - g0],
                    )

            def emit_det(ci):
                nt = CHUNKS[ci]
                A = A_c[ci]
                toff = offs[ci]
                diag = A[:, :, ::33]
                nc.vector.tensor_mul(
                    rowp[:, :nt, :16], diag[:, :, :16], diag[:, :, 16:]
                )
                nc.vector.tensor_mul(
                    rowp[:, :nt, :8], rowp[:, :nt, :8], rowp[:, :nt, 8:16]
                )
                nc.vector.tensor_mul(
                    rowp[:, :nt, :4], rowp[:, :nt, :4], rowp[:, :nt, 4:8]
                )
                nc.vector.tensor_mul(
                    rowp[:, :nt, :2], rowp[:, :nt, :2], rowp[:, :nt, 2:4]
                )
                nc.vector.tensor_mul(
                    detall[:, toff : toff + nt],
                    rowp[:, :nt, 0],
                    rowp[:, :nt, 1],
                )

            # ---- schedule ----
            emit_mlp_chunk(0)
            emit_mlp_chunk(1)
            for k in range(E):
                emit_phase(0, k)
            emit_det(0)
            # chunk-1 ramp; chunk-2 MLP blocks slot between the phases
            c2blocks = list(range(0, CHUNKS[2], MLP_BLK))
            for k in range(RAMP1):
                emit_phase(1, k)
                if k < len(c2blocks):
                    emit_mlp_block(2, c2blocks[k])
            # interleave chunk 1 (ahead by RAMP1) with chunk 2
            for k in range(E):
                if k + RAMP1 < E:
                    emit_phase(1, k + RAMP1)
                emit_phase(2, k)
            emit_det(1)
            emit_det(2)

            # ---- emit dets: [128, 32] -> [32, 128] -> DRAM ----
            psd = ps_t.tile([BC // 128, 128], f32, tag="pst")
            nc.tensor.transpose(psd, detall, ident)
            dsb = consts.tile([BC // 128, 128], f32)
            nc.scalar.copy(dsb, psd)
            nc.sync.dma_start(out[:, :], dsb)

    nsplit = _split_multi_waits(nc)
    if nsplit:
        print(f"[kernel] split {nsplit} surplus sync waits onto NOPs")
    return nc


def _get_nc(include_bias=False):
    key = ("nc", bool(include_bias))
    if key not in _CACHE:
        _CACHE[key] = _build_bass(include_bias)
    return _CACHE[key]


def _first_nonzero_cols(x: np.ndarray) -> np.ndarray:
    """First E column indices of nonzeros of (x == 1) in row-major order."""
    cols = []
    for r in range(x.shape[0]):
        nz = np.flatnonzero(x[r] == 1)
        take = min(E - len(cols), nz.size)
        if take:
            cols.extend(nz[:take].tolist())
        if len(cols) >= E:
            break
    cols = cols[:E] + [0] * (E - len(cols))  # jnp.nonzero(size=E) zero-fill
    return np.asarray(cols, dtype=np.int64)


def kernel(x, W1, b1, W2, b2, W3, b3):
    from concourse import bass_utils

    x = np.ascontiguousarray(np.asarray(x, dtype=np.float32))
    W1 = np.asarray(W1, dtype=np.float32)
    b1 = np.asarray(b1, dtype=np.float32)
    W2 = np.asarray(W2, dtype=np.float32)
    b2 = np.asarray(b2, dtype=np.float32)
    W3 = np.asarray(W3, dtype=np.float32)
    b3 = np.asarray(b3, dtype=np.float32)

    cols = _first_nonzero_cols(x)
    csel = W3[:, cols, :].reshape(H, E * E)
    bsel = b3[cols, :].reshape(1, E * E)
    caug = np.ascontiguousarray(np.concatenate([csel, bsel], axis=0))

    shared = {
        "w1": W1,
        "w2": W2,
        "bias1": b1.reshape(H, 1),
        "bias2": b2.reshape(H, 1),
        "caug": caug,
    }
    in_maps = [
        {"xc": x[c * BC : (c + 1) * BC], **shared} for c in range(NCORES)
    ]

    nc = _get_nc(include_bias=bool(np.any(bsel)))
    res = bass_utils.run_bass_kernel_spmd(nc, in_maps, core_ids=list(range(NCORES)))
    det = np.concatenate(
        [np.asarray(res.results[c]["out"]).reshape(BC) for c in range(NCORES)]
    )
    return det.astype(np.float32)


# revision 15
# speedup vs baseline: 1.1434x; 1.0646x over previous
"""Trainium2 Bass kernel for nn_BACKFLOW (batched backflow determinant).

Math (faithful to the reference):
    cols = first 32 column indices of nonzeros of (x == 1), row-major scan
    h    = tanh(x @ W1 + b1)                       [B, 4]
    h    = tanh(h @ W2 + b2)                       [B, 4]
    S    = tanh(einsum('bf,foe->boe', h, W3) + b3)[:, cols, :]   [B, 32, 32]
    out  = det(S)                                  [B]

Distribution: pure data parallel over the walker (batch) axis across 8
NeuronCores; the tiny MLP params and the selected W3/b3 slices (via `cols`)
are replicated to every core.

Device algorithm per core (4096 walkers = 32 tiles of 128; chunks [2,15,15]):
  * PE: x-tile transposes + W1/W2 matmuls (tanh on ScalarE), then per tile
    S = tanh(h2^T @ C) into SBUF as [128 walkers x tiles x 1024].
  * Batched unblocked LU on VectorE (broadcast APs, fp32), with:
      - adjacent-row pivoting only for k < 8 (swapped row negated so the
        det sign is preserved); smooth reciprocal guard r = piv/(piv^2+c^2).
      - the dominant rank-1 SUBTRACT offloaded for most tiles to a
        PE+ScalarE lane: ScalarE copies the trailing block SBUF->PSUM,
        PE accumulates -I @ tmp onto it (exact fp32), ScalarE copies back.
        VectorE keeps all the outer-product multiplies, a 1-2 row "strip"
        sub per tile (so the next step's pivot row is ready early), and
        the subs for the remaining tiles.
      - the two big chunks' LU loops are interleaved step-by-step so one
        chunk's PE/ScalarE lane hides under the other chunk's VectorE phase.
  * det = tree-product of the final diagonal; one transpose + DMA out.
fp32 everywhere: 16-bit LU storage and fp32r matmuls both fail the accuracy
budget (update-term rounding is amplified by elimination growth).
"""

import sys

if "/opt/trn_rl_repo" not in sys.path:
    sys.path.insert(0, "/opt/trn_rl_repo")

import numpy as np

NCORES = 8
B = 32768
O = 128          # orbitals
E = 32           # electrons == slater matrix size
H = 4            # MLP hidden
BC = B // NCORES     # walkers per core
PIV_CLAMP = 1e-6
NEIGH_UNTIL = 8      # adjacent-row pivoting for k < 8
CHUNKS = [2, 15, 15]
MLP_BLK = 5          # tiles per MLP block
WV = 5               # tiles per LU wave (tmp buffer granularity)
RAMP1 = 8            # chunk-1 head start over chunk-2 (steps)

_CACHE = {}


def _npe_for(ci, k, nt):
    """How many trailing tiles of chunk ci get the PE+ScalarE sub lane."""
    if ci == 0:
        return 0
    return 0  # PE+ScalarE sub lane disabled: the Tile dep tracker misses
    # some strided cb->DVE edges, giving flaky cold-start corruption


def _patch_tile_tail_drain():
    """The tail drain TileContext emits carries >1 sem wait; this walrus
    build only accepts one sync wait per TPB_CTRL drain.  Split them."""
    import concourse.mybir as mybir
    import concourse.tile as tile_mod
    from concourse.tile import TileContext

    if getattr(TileContext, "_drain_patched", False):
        return
    _ScopedClock = tile_mod.ScopedClock

    def _patched(self, tick_clock, wait_clock):
        drain_inst = self.nc.sync.drain()
        wait_clock.add_sem_waits(
            drain_inst.ins, _ScopedClock({None: tick_clock.global_clock})
        )
        si = drain_inst.ins.sync_info
        if si is not None and len(si.on_wait) > 1:
            waits = list(si.on_wait)
            drain_inst.ins.sync_info = mybir.SyncInfo(
                on_wait=waits[:1], on_update=list(si.on_update)
            )
            for i in range(1, len(waits)):
                d2 = self.nc.sync.drain()
                d2.ins.sync_info = mybir.SyncInfo(on_wait=[waits[i]], on_update=[])
        self.nc.all_engine_barrier()
        assert self.sems is not None
        popped = self.nc._tile_sem_poison_stack.pop()
        assert popped is self._sem_poison
        self.nc.clear_and_free_semaphores(list(self.sems.allocated().values()))
        self.nc.all_engine_barrier()

    TileContext._drain_and_barrier = _patched
    TileContext._drain_patched = True


def _split_multi_waits(nc):
    """This walrus build accepts at most one sync-wait command per TPB
    instruction.  Move surplus waits onto same-engine NOPs inserted right
    before the owning instruction."""
    import concourse.mybir as mybir

    count = 0
    for blk in nc.m.functions[0].blocks:
        insts = list(blk.instructions)
        out = []
        changed = False
        for inst in insts:
            si = inst.sync_info
            if si is not None and len(si.on_wait) > 1:
                waits = list(si.on_wait)
                for w in waits[:-1]:
                    count += 1
                    nop = mybir.InstNoOp(
                        name=f"Wsplit-{count}", engine=inst.engine
                    )
                    nop.sync_info = mybir.SyncInfo(on_wait=[w], on_update=[])
                    out.append(nop)
                inst.sync_info = mybir.SyncInfo(
                    on_wait=[waits[-1]], on_update=list(si.on_update)
                )
                changed = True
            out.append(inst)
        if changed:
            blk.instructions = out
    return count


def _build_bass(include_bias):
    import concourse.bass as bass
    import concourse.mybir as mybir
    from concourse.masks import make_identity
    from concourse.tile import TileContext

    _patch_tile_tail_drain()

    f32 = mybir.dt.float32
    u32 = mybir.dt.uint32
    Alu = mybir.AluOpType
    Act = mybir.ActivationFunctionType

    nc = bass.Bass()
    xc = nc.dram_tensor("xc", [BC, O], f32, kind="ExternalInput")
    w1 = nc.dram_tensor("w1", [O, H], f32, kind="ExternalInput")
    w2 = nc.dram_tensor("w2", [H, H], f32, kind="ExternalInput")
    bias1 = nc.dram_tensor("bias1", [H, 1], f32, kind="ExternalInput")
    bias2 = nc.dram_tensor("bias2", [H, 1], f32, kind="ExternalInput")
    caug = nc.dram_tensor("caug", [H + 1, E * E], f32, kind="ExternalInput")
    out = nc.dram_tensor("out", [BC // 128, 128], f32, kind="ExternalOutput")

    NTX = max(CHUNKS)
    CC2 = PIV_CLAMP * PIV_CLAMP
    MMF = 512  # PSUM accumulation-group limit per matmul

    with TileContext(nc) as tc:
        with (
            tc.tile_pool(name="consts", bufs=1) as consts,
            tc.tile_pool(name="mlp", bufs=2) as mlp,
            tc.tile_pool(name="apool", bufs=1) as apool,
            tc.tile_pool(name="work", bufs=1) as work,
            tc.tile_pool(name="wave", bufs=2) as wave,
            tc.tile_pool(name="strip", bufs=2) as strip,
            tc.tile_pool(name="lane", bufs=2, space="PSUM") as lane,
            tc.tile_pool(name="ps_m", bufs=1, space="PSUM") as ps_m,
            tc.tile_pool(name="ps_t", bufs=1, space="PSUM") as ps_t,
            tc.tile_pool(name="ps_h", bufs=1, space="PSUM") as ps_h,
        ):
            ident = consts.tile([128, 128], f32)
            make_identity(nc, ident)
            identn = consts.tile([128, 128], f32)
            nc.vector.tensor_scalar_mul(identn, ident, -1.0)
            # 1-row zero stationary: "+0" fence matmuls register the PSUM
            # accumulate as a tracked write (the start=False MM alone is
            # invisible to the dep tracker -> copy-back raced it)
            zrow = consts.tile([1, 128], f32)
            nc.vector.memset(zrow, 0.0)
            zmov = consts.tile([1, 1], f32)
            nc.vector.memset(zmov, 0.0)
            w1t = consts.tile([O, H], f32)
            nc.sync.dma_start(w1t, w1[:, :])
            w2t = consts.tile([H, H], f32)
            nc.sync.dma_start(w2t, w2[:, :])
            b1t = consts.tile([H, 1], f32)
            nc.sync.dma_start(b1t, bias1[:, :])
            b2t = consts.tile([H, 1], f32)
            nc.sync.dma_start(b2t, bias2[:, :])
            cgt = consts.tile([H, E * E], f32)
            nc.sync.dma_start(cgt, caug[0:H, :])
            if include_bias:
                b3r = consts.tile([1, E * E], f32)
                nc.sync.dma_start(b3r, caug[H : H + 1, :])
                onesr = consts.tile([1, 128], f32)
                nc.vector.memset(onesr, 1.0)

            detall = consts.tile([128, BC // 128], f32)
            guard_s = [consts.tile([128, 1], f32, tag=f"gs{c}", name=f"gs{c}")
                       for c in range(len(CHUNKS))]
            guard_d = [consts.tile([128, 1], f32, tag=f"gd{c}", name=f"gd{c}")
                       for c in range(len(CHUNKS))]
            had_lane = [False] * len(CHUNKS)

            # per-chunk S tiles
            A_c = [apool.tile([128, nt, E * E], f32, tag=f"A{ci}", name=f"A{ci}")
                   for ci, nt in enumerate(CHUNKS)]
            A4_c = [A.rearrange("p t (i j) -> p t i j", i=E) for A in A_c]
            offs = [sum(CHUNKS[:ci]) for ci in range(len(CHUNKS))]

            # persistent LU scratch
            rcp = work.tile([128, NTX], f32)
            pv2 = work.tile([128, NTX], f32)
            nsq = work.tile([128, NTX, 2], f32)
            maskU = work.tile([128, NTX], u32)
            rowp = work.tile([128, NTX, E], f32)
            trow = work.tile([128, NTX, E], f32)

            def emit_mlp_chunk(ci):
                for b0 in range(0, CHUNKS[ci], MLP_BLK):
                    emit_mlp_block(ci, b0)

            def emit_mlp_block(ci, b0):
                nt = CHUNKS[ci]
                toff = offs[ci]
                A = A_c[ci]
                if True:
                    bt = min(MLP_BLK, nt - b0)
                    bw = bt * 128
                    w0 = (toff + b0) * 128
                    xx = mlp.tile([128, MLP_BLK, O], f32, tag="xx")
                    nc.sync.dma_start(
                        xx[:, :bt],
                        xc[w0 : w0 + bw, :].rearrange("(t p) o -> p t o", p=128),
                    )
                    xT = mlp.tile([O, MLP_BLK, 128], f32, tag="xT")
                    for t in range(bt):
                        pst = ps_t.tile([128, 128], f32, tag="pst")
                        nc.tensor.transpose(pst, xx[:, t, :], ident)
                        nc.scalar.copy(xT[:, t, :], pst)

                    xTf = xT.rearrange("p t w -> p (t w)")
                    h1 = mlp.tile([H, MLP_BLK * 128], f32, tag="h1")
                    for s0 in range(0, bw, 512):
                        sl = min(512, bw - s0)
                        ph = ps_h.tile([H, 512], f32, tag="ph")
                        nc.tensor.matmul(ph[:, :sl], w1t, xTf[:, s0 : s0 + sl])
                        nc.scalar.activation(
                            h1[:, s0 : s0 + sl], ph[:, :sl], Act.Tanh, bias=b1t
                        )
                    h2a = mlp.tile([H, MLP_BLK * 128], f32, tag="h2a")
                    for s0 in range(0, bw, 512):
                        sl = min(512, bw - s0)
                        ph2 = ps_h.tile([H, 512], f32, tag="ph")
                        nc.tensor.matmul(ph2[:, :sl], w2t, h1[:, s0 : s0 + sl])
                        nc.scalar.activation(
                            h2a[0:H, s0 : s0 + sl], ph2[:, :sl], Act.Tanh, bias=b2t
                        )
                    for t in range(bt):
                        pm = ps_m.tile([128, E * E], f32, tag="pm")
                        for s in range(2):
                            nc.tensor.matmul(
                                pm[:, s * 512 : (s + 1) * 512],
                                h2a[:, t * 128 : (t + 1) * 128],
                                cgt[:, s * 512 : (s + 1) * 512],
                                start=True,
                                stop=not include_bias,
                            )
                            if include_bias:
                                nc.tensor.matmul(
                                    pm[:, s * 512 : (s + 1) * 512],
                                    onesr,
                                    b3r[:, s * 512 : (s + 1) * 512],
                                    start=False,
                                    stop=True,
                                )
                        nc.scalar.activation(A[:, b0 + t, :], pm, Act.Tanh)

            def emit_phase(ci, k):
                """One LU step k for chunk ci."""
                nt = CHUNKS[ci]
                A = A_c[ci]
                A4 = A4_c[ci]
                if k < NEIGH_UNTIL and k < E - 1:
                    L = E - k
                    pcand = A[:, :, k * 33 : k * 33 + 33 : 32]
                    nc.vector.tensor_mul(nsq[:, :nt], pcand, pcand)
                    nc.vector.tensor_tensor(
                        maskU[:, :nt], nsq[:, :nt, 1], nsq[:, :nt, 0], Alu.is_gt
                    )
                    mb = maskU[:, :nt, None].broadcast_to([128, nt, L])
                    rK = A4[:, :, k, k:]
                    rK1 = A4[:, :, k + 1, k:]
                    nc.vector.tensor_scalar_mul(trow[:, :nt, :L], rK, -1.0)
                    nc.vector.copy_predicated(rK, mb, rK1)
                    nc.vector.copy_predicated(rK1, mb, trow[:, :nt, :L])

                if k >= E - 1:
                    return
                n = E - 1 - k
                piv = A4[:, :, k, k]
                # r = piv / (piv^2 + c^2): smooth sign-correct guarded recip
                nc.vector.tensor_mul(pv2[:, :nt], piv, piv)
                nc.vector.tensor_scalar(
                    pv2[:, :nt], pv2[:, :nt], CC2, None, Alu.add
                )
                nc.vector.reciprocal(pv2[:, :nt], pv2[:, :nt])
                nc.vector.tensor_mul(rcp[:, :nt], piv, pv2[:, :nt])
                row = A4[:, :, k, k + 1 :]
                nc.vector.tensor_mul(
                    rowp[:, :nt, :n],
                    row,
                    rcp[:, :nt, None].broadcast_to([128, nt, n]),
                )

                npe = _npe_for(ci, k, nt)
                srows = 2 if (k + 1) < NEIGH_UNTIL else 1
                if n <= srows:
                    npe = 0
                ndve = nt - npe
                if had_lane[ci]:
                    # gate this chunk's DVE work on its previous phase's
                    # ScalarE copy-backs (the tracker misses some strided
                    # cb->DVE RAW edges; ScE is in-order so the marker op
                    # emitted after those cbs covers them all)
                    nc.vector.tensor_copy(guard_d[ci], guard_s[ci])
                    had_lane[ci] = False

                if npe:
                    # strip rows k+1..k+srows of the PE tiles on DVE so the
                    # next step's pivot row / chain is ready early
                    tws = strip.tile([128, NTX, 2, E], f32, tag="ts")
                    scol = A4[:, ndve:, k + 1 : k + 1 + srows, k]
                    nc.vector.tensor_mul(
                        tws[:, :npe, :srows, :n],
                        scol[:, :, :, None].broadcast_to([128, npe, srows, n]),
                        rowp[:, ndve:, None, :n].broadcast_to(
                            [128, npe, srows, n]
                        ),
                    )
                    nc.vector.tensor_sub(
                        A4[:, ndve:, k + 1 : k + 1 + srows, k + 1 :],
                        A4[:, ndve:, k + 1 : k + 1 + srows, k + 1 :],
                        tws[:, :npe, :srows, :n],
                    )
                # PE-lane tiles first so the lane starts early
                nr = n - srows
                for g0 in range(ndve, nt, WV):
                    g1 = min(g0 + WV, nt)
                    wn = g1 - g0
                    tw = wave.tile([128, WV, nr, n], f32, tag="tw")
                    col = A4[:, g0:g1, k + 1 + srows :, k]
                    nc.vector.tensor_mul(
                        tw[:, :wn],
                        col[:, :, :, None].broadcast_to([128, wn, nr, n]),
                        rowp[:, g0:g1, None, :n].broadcast_to([128, wn, nr, n]),
                    )
                    twf = tw.rearrange("p t i j -> p t (i j)")
                    region = nr * n
                    for t in range(g0, g1):
                        lt = lane.tile([128, 1024], f32, tag="lt")
                        nc.scalar.copy(
                            lt[:, :region].rearrange("p (i j) -> p i j", i=nr),
                            A4[:, t, k + 1 + srows :, k + 1 :],
                        )
                        for s0 in range(0, region, MMF):
                            sl = min(MMF, region - s0)
                            nc.tensor.matmul(
                                lt[:, s0 : s0 + sl],
                                identn,
                                twf[:, t - g0, s0 : s0 + sl],
                                start=False,
                                stop=True,
                                skip_group_check=True,
                            )
                        nc.tensor.matmul(
                            lt[:, 0:1], zrow, zmov,
                            start=False, stop=True, skip_group_check=True,
                        )
                        nc.scalar.copy(
                            A4[:, t, k + 1 + srows :, k + 1 :],
                            lt[:, :region].rearrange("p (i j) -> p i j", i=nr),
                        )
                if npe:
                    nc.scalar.copy(guard_s[ci], identn[:, 0:1])
                    had_lane[ci] = True
                # DVE tiles
                for g0 in range(0, ndve, WV):
                    g1 = min(g0 + WV, ndve)
                    wn = g1 - g0
                    tw = wave.tile([128, WV, n, n], f32, tag="tw")
                    col = A4[:, g0:g1, k + 1 :, k]
                    nc.vector.tensor_mul(
                        tw[:, :wn],
                        col[:, :, :, None].broadcast_to([128, wn, n, n]),
                        rowp[:, g0:g1, None, :n].broadcast_to([128, wn, n, n]),
                    )
                    nc.vector.tensor_sub(
                        A4[:, g0:g1, k + 1 :, k + 1 :],
                        A4[:, g0:g1, k + 1 :, k + 1 :],
                        tw[:, : g1 - g0],
                    )

            def emit_det(ci):
                nt = CHUNKS[ci]
                A = A_c[ci]
                toff = offs[ci]
                diag = A[:, :, ::33]
                nc.vector.tensor_mul(
                    rowp[:, :nt, :16], diag[:, :, :16], diag[:, :, 16:]
                )
                nc.vector.tensor_mul(
                    rowp[:, :nt, :8], rowp[:, :nt, :8], rowp[:, :nt, 8:16]
                )
                nc.vector.tensor_mul(
                    rowp[:, :nt, :4], rowp[:, :nt, :4], rowp[:, :nt, 4:8]
                )
                nc.vector.tensor_mul(
                    rowp[:, :nt, :2], rowp[:, :nt, :2], rowp[:, :nt, 2:4]
                )
                nc.vector.tensor_mul(
                    detall[:, toff : toff + nt],
                    rowp[:, :nt, 0],
                    rowp[:, :nt, 1],
                )

            # ---- schedule ----
            emit_mlp_chunk(0)
            emit_mlp_chunk(1)
            for k in range(E):
                emit_phase(0, k)
            emit_det(0)
            # chunk-1 ramp; chunk-2 MLP blocks slot between the phases
            c2blocks = list(range(0, CHUNKS[2], MLP_BLK))
            for k in range(RAMP1):
                emit_phase(1, k)
                if k < len(c2blocks):
                    emit_mlp_block(2, c2blocks[k])
            # interleave chunk 1 (ahead by RAMP1) with chunk 2
            for k in range(E):
                if k + RAMP1 < E:
                    emit_phase(1, k + RAMP1)
                emit_phase(2, k)
            emit_det(1)
            emit_det(2)

            # ---- emit dets: [128, 32] -> [32, 128] -> DRAM ----
            psd = ps_t.tile([BC // 128, 128], f32, tag="pst")
            nc.tensor.transpose(psd, detall, ident)
            dsb = consts.tile([BC // 128, 128], f32)
            nc.scalar.copy(dsb, psd)
            nc.sync.dma_start(out[:, :], dsb)

    nsplit = _split_multi_waits(nc)
    if nsplit:
        print(f"[kernel] split {nsplit} surplus sync waits onto NOPs")
    return nc


def _get_nc(include_bias=False):
    key = ("nc", bool(include_bias))
    if key not in _CACHE:
        _CACHE[key] = _build_bass(include_bias)
    return _CACHE[key]


def _first_nonzero_cols(x: np.ndarray) -> np.ndarray:
    """First E column indices of nonzeros of (x == 1) in row-major order."""
    cols = []
    for r in range(x.shape[0]):
        nz = np.flatnonzero(x[r] == 1)
        take = min(E - len(cols), nz.size)
        if take:
            cols.extend(nz[:take].tolist())
        if len(cols) >= E:
            break
    cols = cols[:E] + [0] * (E - len(cols))  # jnp.nonzero(size=E) zero-fill
    return np.asarray(cols, dtype=np.int64)


def kernel(x, W1, b1, W2, b2, W3, b3):
    from concourse import bass_utils

    x = np.ascontiguousarray(np.asarray(x, dtype=np.float32))
    W1 = np.asarray(W1, dtype=np.float32)
    b1 = np.asarray(b1, dtype=np.float32)
    W2 = np.asarray(W2, dtype=np.float32)
    b2 = np.asarray(b2, dtype=np.float32)
    W3 = np.asarray(W3, dtype=np.float32)
    b3 = np.asarray(b3, dtype=np.float32)

    cols = _first_nonzero_cols(x)
    csel = W3[:, cols, :].reshape(H, E * E)
    bsel = b3[cols, :].reshape(1, E * E)
    caug = np.ascontiguousarray(np.concatenate([csel, bsel], axis=0))

    shared = {
        "w1": W1,
        "w2": W2,
        "bias1": b1.reshape(H, 1),
        "bias2": b2.reshape(H, 1),
        "caug": caug,
    }
    in_maps = [
        {"xc": x[c * BC : (c + 1) * BC], **shared} for c in range(NCORES)
    ]

    nc = _get_nc(include_bias=bool(np.any(bsel)))
    res = bass_utils.run_bass_kernel_spmd(nc, in_maps, core_ids=list(range(NCORES)))
    det = np.concatenate(
        [np.asarray(res.results[c]["out"]).reshape(BC) for c in range(NCORES)]
    )
    return det.astype(np.float32)


# revision 16
# speedup vs baseline: 1.3614x; 1.1906x over previous
"""Trainium2 Bass kernel for nn_BACKFLOW (batched backflow determinant).

Math (faithful to the reference):
    cols = first 32 column indices of nonzeros of (x == 1), row-major scan
    h    = tanh(x @ W1 + b1)                       [B, 4]
    h    = tanh(h @ W2 + b2)                       [B, 4]
    S    = tanh(einsum('bf,foe->boe', h, W3) + b3)[:, cols, :]   [B, 32, 32]
    out  = det(S)                                  [B]

Distribution: pure data parallel over the walker (batch) axis across 8
NeuronCores; the tiny MLP params and the selected W3/b3 slices (via `cols`)
are replicated to every core.

Device algorithm per core (4096 walkers, chunks of [3, 13, 16] 128-walker
tiles; the small first chunk shortens the time to the first LU step):
  * PE: transpose x tiles, W1/W2 matmuls (tanh fused on ScalarE with a
    per-partition bias), then per 128-walker tile S = tanh(h2^T @ C + b3)
    (b3 via a second accumulating matmul against a ones row) into SBUF laid
    out as [128 walkers(partitions) x tiles x 1024(matrix)].
  * VectorE: batched unblocked LU over all walkers in parallel via
    broadcast (stride-0) access patterns, ~1 elem/lane/cycle.  Stability:
    adjacent-row pivoting for k < 24 where the displaced row is negated so
    the determinant sign is preserved, plus a smooth reciprocal guard
    r = piv / max(piv^2, clamp^2).  The diagonal is never touched after its
    step, so det = tree-product of the final diagonal (5 ops per chunk).
  * One final PE transpose emits dets as [32, 128] for a contiguous DMA out.
GPSIMD offload was measured net-negative: it shares the SBUF port with
VectorE, degrading the dominant tensor_tensor streams from ~1.04 to ~1.7
cycles/element.
"""

import sys

if "/opt/trn_rl_repo" not in sys.path:
    sys.path.insert(0, "/opt/trn_rl_repo")

import numpy as np

NCORES = 8
B = 32768
O = 128          # orbitals
E = 32           # electrons == slater matrix size
H = 4            # MLP hidden
BC = B // NCORES     # walkers per core
NCHUNK = 4
CW = BC // NCHUNK    # walkers per chunk
NT = CW // 128       # 128-walker tiles per chunk
PIV_CLAMP = 1e-6
NEIGHBOR_PIVOT = True
NEIGH_UNTIL = 8       # adjacent-row pivoting for k < 8 (tail clamp-only)

CHUNKS = [3, 13, 16]   # tiles per chunk; small first chunk hides MLP startup
GRP = 8                # big-op tile group (bounds tmp scratch)


def _gp_tiles(k, nt):
    """How many leading tiles of the rank-1 subtract go to GPSIMD.
    GPSIMD has ~2.5us fixed dispatch per op, so only steps with large
    trailing blocks are worth offloading."""
    return 0  # GPSIMD steals DVE's SBUF port; offload is net-negative


_CACHE = {}


def _patch_tile_tail_drain():
    """The tail drain TileContext emits carries >1 sem wait; this walrus
    build only accepts one sync wait per TPB_CTRL drain.  Split them."""
    import concourse.mybir as mybir
    import concourse.tile as tile_mod
    from concourse.tile import TileContext

    if getattr(TileContext, "_drain_patched", False):
        return
    _ScopedClock = tile_mod.ScopedClock

    def _patched(self, tick_clock, wait_clock):
        drain_inst = self.nc.sync.drain()
        wait_clock.add_sem_waits(
            drain_inst.ins, _ScopedClock({None: tick_clock.global_clock})
        )
        si = drain_inst.ins.sync_info
        if si is not None and len(si.on_wait) > 1:
            waits = list(si.on_wait)
            drain_inst.ins.sync_info = mybir.SyncInfo(
                on_wait=waits[:1], on_update=list(si.on_update)
            )
            for i in range(1, len(waits)):
                d2 = self.nc.sync.drain()
                d2.ins.sync_info = mybir.SyncInfo(on_wait=[waits[i]], on_update=[])
        self.nc.all_engine_barrier()
        assert self.sems is not None
        popped = self.nc._tile_sem_poison_stack.pop()
        assert popped is self._sem_poison
        self.nc.clear_and_free_semaphores(list(self.sems.allocated().values()))
        self.nc.all_engine_barrier()

    TileContext._drain_and_barrier = _patched
    TileContext._drain_patched = True


def _split_multi_waits(nc):
    """This walrus build accepts at most one sync-wait command per TPB
    instruction.  Move surplus waits onto same-engine NOPs inserted right
    before the owning instruction."""
    import concourse.mybir as mybir

    count = 0
    for blk in nc.m.functions[0].blocks:
        insts = list(blk.instructions)
        out = []
        changed = False
        for inst in insts:
            si = inst.sync_info
            if si is not None and len(si.on_wait) > 1:
                waits = list(si.on_wait)
                for w in waits[:-1]:
                    count += 1
                    nop = mybir.InstNoOp(
                        name=f"Wsplit-{count}", engine=inst.engine
                    )
                    nop.sync_info = mybir.SyncInfo(on_wait=[w], on_update=[])
                    out.append(nop)
                inst.sync_info = mybir.SyncInfo(
                    on_wait=[waits[-1]], on_update=list(si.on_update)
                )
                changed = True
            out.append(inst)
        if changed:
            blk.instructions = out
    return count


def _build_bass(include_bias):
    import concourse.bass as bass
    import concourse.mybir as mybir
    from concourse.masks import make_identity
    from concourse.tile import TileContext

    _patch_tile_tail_drain()

    f32 = mybir.dt.float32
    u32 = mybir.dt.uint32
    Alu = mybir.AluOpType
    Act = mybir.ActivationFunctionType

    nc = bass.Bass()
    xc = nc.dram_tensor("xc", [BC, O], f32, kind="ExternalInput")
    w1 = nc.dram_tensor("w1", [O, H], f32, kind="ExternalInput")
    w2 = nc.dram_tensor("w2", [H, H], f32, kind="ExternalInput")
    bias1 = nc.dram_tensor("bias1", [H, 1], f32, kind="ExternalInput")
    bias2 = nc.dram_tensor("bias2", [H, 1], f32, kind="ExternalInput")
    caug = nc.dram_tensor("caug", [H + 1, E * E], f32, kind="ExternalInput")
    out = nc.dram_tensor("out", [BC // 128, 128], f32, kind="ExternalOutput")

    with TileContext(nc) as tc:
        with (
            tc.tile_pool(name="consts", bufs=1) as consts,
            tc.tile_pool(name="mlp", bufs=2) as mlp,
            tc.tile_pool(name="apool", bufs=2) as apool,
            tc.tile_pool(name="work", bufs=1) as work,
            tc.tile_pool(name="ps_t", bufs=2, space="PSUM") as ps_t,
            tc.tile_pool(name="ps_m", bufs=2, space="PSUM") as ps_m,
        ):
            ident = consts.tile([128, 128], f32)
            make_identity(nc, ident)
            w1t = consts.tile([O, H], f32)
            nc.sync.dma_start(w1t, w1[:, :])
            w2t = consts.tile([H, H], f32)
            nc.sync.dma_start(w2t, w2[:, :])
            b1t = consts.tile([H, 1], f32)
            nc.sync.dma_start(b1t, bias1[:, :])
            b2t = consts.tile([H, 1], f32)
            nc.sync.dma_start(b2t, bias2[:, :])
            cgt = consts.tile([H, E * E], f32)
            nc.sync.dma_start(cgt, caug[0:H, :])
            if include_bias:
                b3r = consts.tile([1, E * E], f32)
                nc.sync.dma_start(b3r, caug[H : H + 1, :])
                onesr = consts.tile([1, 128], f32)
                nc.vector.memset(onesr, 1.0)

            detall = consts.tile([128, BC // 128], f32)

            # persistent LU scratch (sized for the largest chunk)
            NTX = max(CHUNKS)
            rcp = work.tile([128, NTX], f32)
            pv2 = work.tile([128, NTX], f32)
            nsq = work.tile([128, NTX, 2], f32)
            maskU = work.tile([128, NTX], u32)
            rowp = work.tile([128, NTX, E], f32)
            trow = work.tile([128, NTX, E], f32)
            tmp = work.tile([128, GRP, E - 1, E - 1], f32)

            toff = 0
            for c, nt in enumerate(CHUNKS):
                # ---- MLP in blocks of <= 8 tiles ----
                A = apool.tile([128, nt, E * E], f32, tag="A")
                for b0 in range(0, nt, 8):
                    bt = min(8, nt - b0)
                    bw = bt * 128
                    w0 = (toff + b0) * 128
                    xx = mlp.tile([128, bt, O], f32, tag="xx")
                    nc.sync.dma_start(
                        xx,
                        xc[w0 : w0 + bw, :].rearrange("(t p) o -> p t o", p=128),
                    )
                    xT = mlp.tile([O, bt, 128], f32, tag="xT")
                    for t in range(bt):
                        pst = ps_t.tile([128, 128], f32, tag="pst")
                        nc.tensor.transpose(pst, xx[:, t, :], ident)
                        nc.scalar.copy(xT[:, t, :], pst)

                    xTf = xT.rearrange("p t w -> p (t w)")
                    h1 = mlp.tile([H, bw], f32, tag="h1")
                    for s0 in range(0, bw, 512):
                        sl = min(512, bw - s0)
                        ph = ps_t.tile([H, 512], f32, tag="ph")
                        nc.tensor.matmul(ph[:, :sl], w1t, xTf[:, s0 : s0 + sl])
                        nc.scalar.activation(
                            h1[:, s0 : s0 + sl], ph[:, :sl], Act.Tanh, bias=b1t
                        )
                    h2a = mlp.tile([H, bw], f32, tag="h2a")
                    for s0 in range(0, bw, 512):
                        sl = min(512, bw - s0)
                        ph2 = ps_t.tile([H, 512], f32, tag="ph")
                        nc.tensor.matmul(ph2[:, :sl], w2t, h1[:, s0 : s0 + sl])
                        nc.scalar.activation(
                            h2a[0:H, s0 : s0 + sl], ph2[:, :sl], Act.Tanh, bias=b2t
                        )
                    for t in range(bt):
                        pm = ps_m.tile([128, E * E], f32, tag="pm")
                        for s in range(2):
                            nc.tensor.matmul(
                                pm[:, s * 512 : (s + 1) * 512],
                                h2a[:, t * 128 : (t + 1) * 128],
                                cgt[:, s * 512 : (s + 1) * 512],
                                start=True,
                                stop=not include_bias,
                            )
                            if include_bias:
                                nc.tensor.matmul(
                                    pm[:, s * 512 : (s + 1) * 512],
                                    onesr,
                                    b3r[:, s * 512 : (s + 1) * 512],
                                    start=False,
                                    stop=True,
                                )
                        nc.scalar.activation(A[:, b0 + t, :], pm, Act.Tanh)

                # ---- batched LU (no transpose; walkers on partitions) ----
                # Swaps negate the displaced row, so det needs no sign
                # bookkeeping; the diagonal is never touched after its step,
                # so det = product of the final diagonal.
                A4 = A.rearrange("p t (i j) -> p t i j", i=E)
                for k in range(E):
                    if NEIGHBOR_PIVOT and k < NEIGH_UNTIL and k < E - 1:
                        L = E - k
                        pcand = A[:, :, k * 33 : k * 33 + 33 : 32]
                        nc.vector.tensor_mul(nsq[:, :nt], pcand, pcand)
                        nc.vector.tensor_tensor(
                            maskU[:, :nt], nsq[:, :nt, 1], nsq[:, :nt, 0], Alu.is_gt
                        )
                        mb = maskU[:, :nt, None].broadcast_to([128, nt, L])
                        rK = A4[:, :, k, k:]
                        rK1 = A4[:, :, k + 1, k:]
                        nc.vector.tensor_scalar_mul(trow[:, :nt, :L], rK, -1.0)
                        nc.vector.copy_predicated(rK, mb, rK1)
                        nc.vector.copy_predicated(rK1, mb, trow[:, :nt, :L])

                    if k < E - 1:
                        piv = A4[:, :, k, k]
                        # r = piv / max(piv^2, clamp^2): 1/piv with a smooth,
                        # sign-correct guard near zero
                        nc.vector.tensor_mul(pv2[:, :nt], piv, piv)
                        nc.vector.tensor_scalar(
                            pv2[:, :nt], pv2[:, :nt], PIV_CLAMP * PIV_CLAMP,
                            None, Alu.max,
                        )
                        nc.vector.reciprocal(pv2[:, :nt], pv2[:, :nt])
                        nc.vector.tensor_mul(rcp[:, :nt], piv, pv2[:, :nt])
                        n = E - 1 - k
                        row = A4[:, :, k, k + 1 :]
                        nc.vector.tensor_mul(
                            rowp[:, :nt, :n],
                            row,
                            rcp[:, :nt, None].broadcast_to([128, nt, n]),
                        )
                        ngp = _gp_tiles(k, nt)
                        for g0 in range(0, nt, GRP):
                            gn = min(GRP, nt - g0)
                            col = A4[:, g0 : g0 + gn, k + 1 :, k]
                            nc.vector.tensor_mul(
                                tmp[:, :gn, :n, :n],
                                col[:, :, :, None].broadcast_to([128, gn, n, n]),
                                rowp[:, g0 : g0 + gn, None, :n].broadcast_to(
                                    [128, gn, n, n]
                                ),
                            )
                            # GPSIMD owns the leading ngp tiles so its sub
                            # starts right after the first outer group
                            for eng, t0, t1 in (
                                (nc.gpsimd, g0, min(g0 + gn, ngp)),
                                (nc.vector, max(g0, ngp), g0 + gn),
                            ):
                                if t0 >= t1:
                                    continue
                                eng.tensor_sub(
                                    A4[:, t0:t1, k + 1 :, k + 1 :],
                                    A4[:, t0:t1, k + 1 :, k + 1 :],
                                    tmp[:, t0 - g0 : t1 - g0, :n, :n],
                                )

                # det = product over the diagonal (tree reduce)
                diag = A[:, :, ::33]
                nc.vector.tensor_mul(
                    rowp[:, :nt, :16], diag[:, :, :16], diag[:, :, 16:]
                )
                nc.vector.tensor_mul(
                    rowp[:, :nt, :8], rowp[:, :nt, :8], rowp[:, :nt, 8:16]
                )
                nc.vector.tensor_mul(
                    rowp[:, :nt, :4], rowp[:, :nt, :4], rowp[:, :nt, 4:8]
                )
                nc.vector.tensor_mul(
                    rowp[:, :nt, :2], rowp[:, :nt, :2], rowp[:, :nt, 2:4]
                )
                nc.vector.tensor_mul(
                    detall[:, toff : toff + nt],
                    rowp[:, :nt, 0],
                    rowp[:, :nt, 1],
                )
                toff += nt

            # ---- emit dets: [128, 32] -> [32, 128] -> DRAM ----
            psd = ps_t.tile([BC // 128, 128], f32, tag="ph")
            nc.tensor.transpose(psd, detall, ident)
            dsb = consts.tile([BC // 128, 128], f32)
            nc.scalar.copy(dsb, psd)
            nc.sync.dma_start(out[:, :], dsb)

    nsplit = _split_multi_waits(nc)
    if nsplit:
        print(f"[kernel] split {nsplit} surplus sync waits onto NOPs")
    return nc


def _get_nc(include_bias=False):
    key = ("nc", bool(include_bias))
    if key not in _CACHE:
        _CACHE[key] = _build_bass(include_bias)
    return _CACHE[key]


def _first_nonzero_cols(x: np.ndarray) -> np.ndarray:
    """First E column indices of nonzeros of (x == 1) in row-major order."""
    cols = []
    for r in range(x.shape[0]):
        nz = np.flatnonzero(x[r] == 1)
        take = min(E - len(cols), nz.size)
        if take:
            cols.extend(nz[:take].tolist())
        if len(cols) >= E:
            break
    cols = cols[:E] + [0] * (E - len(cols))  # jnp.nonzero(size=E) zero-fill
    return np.asarray(cols, dtype=np.int64)


def kernel(x, W1, b1, W2, b2, W3, b3):
    from concourse import bass_utils

    x = np.ascontiguousarray(np.asarray(x, dtype=np.float32))
    W1 = np.asarray(W1, dtype=np.float32)
    b1 = np.asarray(b1, dtype=np.float32)
    W2 = np.asarray(W2, dtype=np.float32)
    b2 = np.asarray(b2, dtype=np.float32)
    W3 = np.asarray(W3, dtype=np.float32)
    b3 = np.asarray(b3, dtype=np.float32)

    cols = _first_nonzero_cols(x)
    csel = W3[:, cols, :].reshape(H, E * E)
    bsel = b3[cols, :].reshape(1, E * E)
    caug = np.ascontiguousarray(np.concatenate([csel, bsel], axis=0))

    shared = {
        "w1": W1,
        "w2": W2,
        "bias1": b1.reshape(H, 1),
        "bias2": b2.reshape(H, 1),
        "caug": caug,
    }
    in_maps = [
        {"xc": x[c * BC : (c + 1) * BC], **shared} for c in range(NCORES)
    ]

    nc = _get_nc(include_bias=bool(np.any(bsel)))
    res = bass_utils.run_bass_kernel_spmd(nc, in_maps, core_ids=list(range(NCORES)))
    det = np.concatenate(
        [np.asarray(res.results[c]["out"]).reshape(BC) for c in range(NCORES)]
    )
    return det.astype(np.float32)



# revision 17
# speedup vs baseline: 1.4197x; 1.0429x over previous
"""Trainium2 Bass kernel for nn_BACKFLOW (batched backflow determinant).

Math (faithful to the reference):
    cols = first 32 column indices of nonzeros of (x == 1), row-major scan
    h    = tanh(x @ W1 + b1)                       [B, 4]
    h    = tanh(h @ W2 + b2)                       [B, 4]
    S    = tanh(einsum('bf,foe->boe', h, W3) + b3)[:, cols, :]   [B, 32, 32]
    out  = det(S)                                  [B]

Distribution: pure data parallel over the walker (batch) axis across 8
NeuronCores; the tiny MLP params and the selected W3/b3 slices (via `cols`)
are replicated to every core.

Device algorithm per core (4096 walkers, chunks of [3, 13, 16] 128-walker
tiles; the small first chunk shortens the time to the first LU step):
  * PE: transpose x tiles, W1/W2 matmuls (tanh fused on ScalarE with a
    per-partition bias), then per 128-walker tile S = tanh(h2^T @ C + b3)
    (b3 via a second accumulating matmul against a ones row) into SBUF laid
    out as [128 walkers(partitions) x tiles x 1024(matrix)].
  * VectorE: batched unblocked LU over all walkers in parallel via
    broadcast (stride-0) access patterns, ~1 elem/lane/cycle.  Stability:
    adjacent-row pivoting for k < 24 where the displaced row is negated so
    the determinant sign is preserved, plus a smooth reciprocal guard
    r = piv / max(piv^2, clamp^2).  The diagonal is never touched after its
    step, so det = tree-product of the final diagonal (5 ops per chunk).
  * One final PE transpose emits dets as [32, 128] for a contiguous DMA out.
GPSIMD offload was measured net-negative: it shares the SBUF port with
VectorE, degrading the dominant tensor_tensor streams from ~1.04 to ~1.7
cycles/element.
"""

import sys

if "/opt/trn_rl_repo" not in sys.path:
    sys.path.insert(0, "/opt/trn_rl_repo")

import numpy as np

NCORES = 8
B = 32768
O = 128          # orbitals
E = 32           # electrons == slater matrix size
H = 4            # MLP hidden
BC = B // NCORES     # walkers per core
NCHUNK = 4
CW = BC // NCHUNK    # walkers per chunk
NT = CW // 128       # 128-walker tiles per chunk
PIV_CLAMP = 1e-6
NEIGHBOR_PIVOT = True
NEIGH_UNTIL = 0       # clamp-only (no pivoting)

CHUNKS = [3, 13, 16]   # tiles per chunk; small first chunk hides MLP startup
GRP = 8                # big-op tile group (bounds tmp scratch)


def _gp_tiles(k, nt):
    """How many leading tiles of the rank-1 subtract go to GPSIMD.
    GPSIMD has ~2.5us fixed dispatch per op, so only steps with large
    trailing blocks are worth offloading."""
    return 0  # GPSIMD steals DVE's SBUF port; offload is net-negative


_CACHE = {}


def _patch_tile_tail_drain():
    """The tail drain TileContext emits carries >1 sem wait; this walrus
    build only accepts one sync wait per TPB_CTRL drain.  Split them."""
    import concourse.mybir as mybir
    import concourse.tile as tile_mod
    from concourse.tile import TileContext

    if getattr(TileContext, "_drain_patched", False):
        return
    _ScopedClock = tile_mod.ScopedClock

    def _patched(self, tick_clock, wait_clock):
        drain_inst = self.nc.sync.drain()
        wait_clock.add_sem_waits(
            drain_inst.ins, _ScopedClock({None: tick_clock.global_clock})
        )
        si = drain_inst.ins.sync_info
        if si is not None and len(si.on_wait) > 1:
            waits = list(si.on_wait)
            drain_inst.ins.sync_info = mybir.SyncInfo(
                on_wait=waits[:1], on_update=list(si.on_update)
            )
            for i in range(1, len(waits)):
                d2 = self.nc.sync.drain()
                d2.ins.sync_info = mybir.SyncInfo(on_wait=[waits[i]], on_update=[])
        self.nc.all_engine_barrier()
        assert self.sems is not None
        popped = self.nc._tile_sem_poison_stack.pop()
        assert popped is self._sem_poison
        self.nc.clear_and_free_semaphores(list(self.sems.allocated().values()))
        self.nc.all_engine_barrier()

    TileContext._drain_and_barrier = _patched
    TileContext._drain_patched = True


def _split_multi_waits(nc):
    """This walrus build accepts at most one sync-wait command per TPB
    instruction.  Move surplus waits onto same-engine NOPs inserted right
    before the owning instruction."""
    import concourse.mybir as mybir

    count = 0
    for blk in nc.m.functions[0].blocks:
        insts = list(blk.instructions)
        out = []
        changed = False
        for inst in insts:
            si = inst.sync_info
            if si is not None and len(si.on_wait) > 1:
                waits = list(si.on_wait)
                for w in waits[:-1]:
                    count += 1
                    nop = mybir.InstNoOp(
                        name=f"Wsplit-{count}", engine=inst.engine
                    )
                    nop.sync_info = mybir.SyncInfo(on_wait=[w], on_update=[])
                    out.append(nop)
                inst.sync_info = mybir.SyncInfo(
                    on_wait=[waits[-1]], on_update=list(si.on_update)
                )
                changed = True
            out.append(inst)
        if changed:
            blk.instructions = out
    return count


def _build_bass(include_bias):
    import concourse.bass as bass
    import concourse.mybir as mybir
    from concourse.masks import make_identity
    from concourse.tile import TileContext

    _patch_tile_tail_drain()

    f32 = mybir.dt.float32
    u32 = mybir.dt.uint32
    Alu = mybir.AluOpType
    Act = mybir.ActivationFunctionType

    nc = bass.Bass()
    xc = nc.dram_tensor("xc", [BC, O], f32, kind="ExternalInput")
    w1 = nc.dram_tensor("w1", [O, H], f32, kind="ExternalInput")
    w2 = nc.dram_tensor("w2", [H, H], f32, kind="ExternalInput")
    bias1 = nc.dram_tensor("bias1", [H, 1], f32, kind="ExternalInput")
    bias2 = nc.dram_tensor("bias2", [H, 1], f32, kind="ExternalInput")
    caug = nc.dram_tensor("caug", [H + 1, E * E], f32, kind="ExternalInput")
    out = nc.dram_tensor("out", [BC // 128, 128], f32, kind="ExternalOutput")

    with TileContext(nc) as tc:
        with (
            tc.tile_pool(name="consts", bufs=1) as consts,
            tc.tile_pool(name="mlp", bufs=2) as mlp,
            tc.tile_pool(name="apool", bufs=2) as apool,
            tc.tile_pool(name="work", bufs=1) as work,
            tc.tile_pool(name="ps_t", bufs=2, space="PSUM") as ps_t,
            tc.tile_pool(name="ps_m", bufs=2, space="PSUM") as ps_m,
        ):
            ident = consts.tile([128, 128], f32)
            make_identity(nc, ident)
            w1t = consts.tile([O, H], f32)
            nc.sync.dma_start(w1t, w1[:, :])
            w2t = consts.tile([H, H], f32)
            nc.sync.dma_start(w2t, w2[:, :])
            b1t = consts.tile([H, 1], f32)
            nc.sync.dma_start(b1t, bias1[:, :])
            b2t = consts.tile([H, 1], f32)
            nc.sync.dma_start(b2t, bias2[:, :])
            cgt = consts.tile([H, E * E], f32)
            nc.sync.dma_start(cgt, caug[0:H, :])
            if include_bias:
                b3r = consts.tile([1, E * E], f32)
                nc.sync.dma_start(b3r, caug[H : H + 1, :])
                onesr = consts.tile([1, 128], f32)
                nc.vector.memset(onesr, 1.0)

            detall = consts.tile([128, BC // 128], f32)

            # persistent LU scratch (sized for the largest chunk)
            NTX = max(CHUNKS)
            rcp = work.tile([128, NTX], f32)
            pv2 = work.tile([128, NTX], f32)
            nsq = work.tile([128, NTX, 2], f32)
            maskU = work.tile([128, NTX], u32)
            rowp = work.tile([128, NTX, E], f32)
            trow = work.tile([128, NTX, E], f32)
            tmp = work.tile([128, GRP, E - 1, E - 1], f32)

            toff = 0
            for c, nt in enumerate(CHUNKS):
                # ---- MLP in blocks of <= 8 tiles ----
                A = apool.tile([128, nt, E * E], f32, tag="A")
                for b0 in range(0, nt, 8):
                    bt = min(8, nt - b0)
                    bw = bt * 128
                    w0 = (toff + b0) * 128
                    xx = mlp.tile([128, bt, O], f32, tag="xx")
                    nc.sync.dma_start(
                        xx,
                        xc[w0 : w0 + bw, :].rearrange("(t p) o -> p t o", p=128),
                    )
                    xT = mlp.tile([O, bt, 128], f32, tag="xT")
                    for t in range(bt):
                        pst = ps_t.tile([128, 128], f32, tag="pst")
                        nc.tensor.transpose(pst, xx[:, t, :], ident)
                        nc.scalar.copy(xT[:, t, :], pst)

                    xTf = xT.rearrange("p t w -> p (t w)")
                    h1 = mlp.tile([H, bw], f32, tag="h1")
                    for s0 in range(0, bw, 512):
                        sl = min(512, bw - s0)
                        ph = ps_t.tile([H, 512], f32, tag="ph")
                        nc.tensor.matmul(ph[:, :sl], w1t, xTf[:, s0 : s0 + sl])
                        nc.scalar.activation(
                            h1[:, s0 : s0 + sl], ph[:, :sl], Act.Tanh, bias=b1t
                        )
                    h2a = mlp.tile([H, bw], f32, tag="h2a")
                    for s0 in range(0, bw, 512):
                        sl = min(512, bw - s0)
                        ph2 = ps_t.tile([H, 512], f32, tag="ph")
                        nc.tensor.matmul(ph2[:, :sl], w2t, h1[:, s0 : s0 + sl])
                        nc.scalar.activation(
                            h2a[0:H, s0 : s0 + sl], ph2[:, :sl], Act.Tanh, bias=b2t
                        )
                    for t in range(bt):
                        pm = ps_m.tile([128, E * E], f32, tag="pm")
                        for s in range(2):
                            nc.tensor.matmul(
                                pm[:, s * 512 : (s + 1) * 512],
                                h2a[:, t * 128 : (t + 1) * 128],
                                cgt[:, s * 512 : (s + 1) * 512],
                                start=True,
                                stop=not include_bias,
                            )
                            if include_bias:
                                nc.tensor.matmul(
                                    pm[:, s * 512 : (s + 1) * 512],
                                    onesr,
                                    b3r[:, s * 512 : (s + 1) * 512],
                                    start=False,
                                    stop=True,
                                )
                        nc.scalar.activation(A[:, b0 + t, :], pm, Act.Tanh)

                # ---- batched LU (no transpose; walkers on partitions) ----
                # Swaps negate the displaced row, so det needs no sign
                # bookkeeping; the diagonal is never touched after its step,
                # so det = product of the final diagonal.
                A4 = A.rearrange("p t (i j) -> p t i j", i=E)
                for k in range(E):
                    if NEIGHBOR_PIVOT and k < NEIGH_UNTIL and k < E - 1:
                        L = E - k
                        pcand = A[:, :, k * 33 : k * 33 + 33 : 32]
                        nc.vector.tensor_mul(nsq[:, :nt], pcand, pcand)
                        nc.vector.tensor_tensor(
                            maskU[:, :nt], nsq[:, :nt, 1], nsq[:, :nt, 0], Alu.is_gt
                        )
                        mb = maskU[:, :nt, None].broadcast_to([128, nt, L])
                        rK = A4[:, :, k, k:]
                        rK1 = A4[:, :, k + 1, k:]
                        nc.vector.tensor_scalar_mul(trow[:, :nt, :L], rK, -1.0)
                        nc.vector.copy_predicated(rK, mb, rK1)
                        nc.vector.copy_predicated(rK1, mb, trow[:, :nt, :L])

                    if k < E - 1:
                        piv = A4[:, :, k, k]
                        # r = piv / max(piv^2, clamp^2): 1/piv with a smooth,
                        # sign-correct guard near zero
                        nc.vector.tensor_mul(pv2[:, :nt], piv, piv)
                        nc.vector.tensor_scalar(
                            pv2[:, :nt], pv2[:, :nt], PIV_CLAMP * PIV_CLAMP,
                            None, Alu.max,
                        )
                        nc.vector.reciprocal(pv2[:, :nt], pv2[:, :nt])
                        nc.vector.tensor_mul(rcp[:, :nt], piv, pv2[:, :nt])
                        n = E - 1 - k
                        row = A4[:, :, k, k + 1 :]
                        nc.vector.tensor_mul(
                            rowp[:, :nt, :n],
                            row,
                            rcp[:, :nt, None].broadcast_to([128, nt, n]),
                        )
                        ngp = _gp_tiles(k, nt)
                        for g0 in range(0, nt, GRP):
                            gn = min(GRP, nt - g0)
                            col = A4[:, g0 : g0 + gn, k + 1 :, k]
                            nc.vector.tensor_mul(
                                tmp[:, :gn, :n, :n],
                                col[:, :, :, None].broadcast_to([128, gn, n, n]),
                                rowp[:, g0 : g0 + gn, None, :n].broadcast_to(
                                    [128, gn, n, n]
                                ),
                            )
                            # GPSIMD owns the leading ngp tiles so its sub
                            # starts right after the first outer group
                            for eng, t0, t1 in (
                                (nc.gpsimd, g0, min(g0 + gn, ngp)),
                                (nc.vector, max(g0, ngp), g0 + gn),
                            ):
                                if t0 >= t1:
                                    continue
                                eng.tensor_sub(
                                    A4[:, t0:t1, k + 1 :, k + 1 :],
                                    A4[:, t0:t1, k + 1 :, k + 1 :],
                                    tmp[:, t0 - g0 : t1 - g0, :n, :n],
                                )

                # det = product over the diagonal (tree reduce)
                diag = A[:, :, ::33]
                nc.vector.tensor_mul(
                    rowp[:, :nt, :16], diag[:, :, :16], diag[:, :, 16:]
                )
                nc.vector.tensor_mul(
                    rowp[:, :nt, :8], rowp[:, :nt, :8], rowp[:, :nt, 8:16]
                )
                nc.vector.tensor_mul(
                    rowp[:, :nt, :4], rowp[:, :nt, :4], rowp[:, :nt, 4:8]
                )
                nc.vector.tensor_mul(
                    rowp[:, :nt, :2], rowp[:, :nt, :2], rowp[:, :nt, 2:4]
                )
                nc.vector.tensor_mul(
                    detall[:, toff : toff + nt],
                    rowp[:, :nt, 0],
                    rowp[:, :nt, 1],
                )
                toff += nt

            # ---- emit dets: [128, 32] -> [32, 128] -> DRAM ----
            psd = ps_t.tile([BC // 128, 128], f32, tag="ph")
            nc.tensor.transpose(psd, detall, ident)
            dsb = consts.tile([BC // 128, 128], f32)
            nc.scalar.copy(dsb, psd)
            nc.sync.dma_start(out[:, :], dsb)

    nsplit = _split_multi_waits(nc)
    if nsplit:
        print(f"[kernel] split {nsplit} surplus sync waits onto NOPs")
    return nc


def _get_nc(include_bias=False):
    key = ("nc", bool(include_bias))
    if key not in _CACHE:
        _CACHE[key] = _build_bass(include_bias)
    return _CACHE[key]


def _first_nonzero_cols(x: np.ndarray) -> np.ndarray:
    """First E column indices of nonzeros of (x == 1) in row-major order."""
    cols = []
    for r in range(x.shape[0]):
        nz = np.flatnonzero(x[r] == 1)
        take = min(E - len(cols), nz.size)
        if take:
            cols.extend(nz[:take].tolist())
        if len(cols) >= E:
            break
    cols = cols[:E] + [0] * (E - len(cols))  # jnp.nonzero(size=E) zero-fill
    return np.asarray(cols, dtype=np.int64)


def kernel(x, W1, b1, W2, b2, W3, b3):
    from concourse import bass_utils

    x = np.ascontiguousarray(np.asarray(x, dtype=np.float32))
    W1 = np.asarray(W1, dtype=np.float32)
    b1 = np.asarray(b1, dtype=np.float32)
    W2 = np.asarray(W2, dtype=np.float32)
    b2 = np.asarray(b2, dtype=np.float32)
    W3 = np.asarray(W3, dtype=np.float32)
    b3 = np.asarray(b3, dtype=np.float32)

    cols = _first_nonzero_cols(x)
    csel = W3[:, cols, :].reshape(H, E * E)
    bsel = b3[cols, :].reshape(1, E * E)
    caug = np.ascontiguousarray(np.concatenate([csel, bsel], axis=0))

    shared = {
        "w1": W1,
        "w2": W2,
        "bias1": b1.reshape(H, 1),
        "bias2": b2.reshape(H, 1),
        "caug": caug,
    }
    in_maps = [
        {"xc": x[c * BC : (c + 1) * BC], **shared} for c in range(NCORES)
    ]

    nc = _get_nc(include_bias=bool(np.any(bsel)))
    res = bass_utils.run_bass_kernel_spmd(nc, in_maps, core_ids=list(range(NCORES)))
    det = np.concatenate(
        [np.asarray(res.results[c]["out"]).reshape(BC) for c in range(NCORES)]
    )
    return det.astype(np.float32)



# revision 18
# speedup vs baseline: 1.4201x; 1.0003x over previous
"""Trainium2 Bass kernel for nn_BACKFLOW (batched backflow determinant).

Math (faithful to the reference):
    cols = first 32 column indices of nonzeros of (x == 1), row-major scan
    h    = tanh(x @ W1 + b1)                       [B, 4]
    h    = tanh(h @ W2 + b2)                       [B, 4]
    S    = tanh(einsum('bf,foe->boe', h, W3) + b3)[:, cols, :]   [B, 32, 32]
    out  = det(S)                                  [B]

Distribution: pure data parallel over the walker (batch) axis across 8
NeuronCores; the tiny MLP params and the selected W3/b3 slices (via `cols`)
are replicated to every core.

Device algorithm per core (4096 walkers, chunks of [3, 13, 16] 128-walker
tiles; the small first chunk shortens the time to the first LU step):
  * PE: transpose x tiles, W1/W2 matmuls (tanh fused on ScalarE with a
    per-partition bias), then per 128-walker tile S = tanh(h2^T @ C + b3)
    (b3 via a second accumulating matmul against a ones row) into SBUF laid
    out as [128 walkers(partitions) x tiles x 1024(matrix)].
  * VectorE: batched unblocked LU over all walkers in parallel via
    broadcast (stride-0) access patterns, ~1 elem/lane/cycle.  Stability:
    clamp-only (no pivoting): r = piv / max(piv^2, clamp^2) is a smooth,
    sign-correct guard; measured rel err 6e-3 vs the f64 oracle (3x margin
    under the 2e-2 gate; adjacent-row pivoting for k<8 costs +39us and
    measures no better).  The diagonal is never touched after its step, so
    det = tree-product of the final diagonal (5 ops per chunk).
  * One final PE transpose emits dets as [32, 128] for a contiguous DMA out.
GPSIMD offload was measured net-negative: it shares the SBUF port with
VectorE, degrading the dominant tensor_tensor streams from ~1.04 to ~1.7
cycles/element.
"""

import sys

if "/opt/trn_rl_repo" not in sys.path:
    sys.path.insert(0, "/opt/trn_rl_repo")

import numpy as np

NCORES = 8
B = 32768
O = 128          # orbitals
E = 32           # electrons == slater matrix size
H = 4            # MLP hidden
BC = B // NCORES     # walkers per core
NCHUNK = 4
CW = BC // NCHUNK    # walkers per chunk
NT = CW // 128       # 128-walker tiles per chunk
PIV_CLAMP = 1e-6
NEIGHBOR_PIVOT = True
NEIGH_UNTIL = 0       # clamp-only (no pivoting)

CHUNKS = [3, 13, 16]   # tiles per chunk; small first chunk hides MLP startup
GRP = 8                # big-op tile group (bounds tmp scratch)


def _gp_tiles(k, nt):
    """How many leading tiles of the rank-1 subtract go to GPSIMD.
    GPSIMD has ~2.5us fixed dispatch per op, so only steps with large
    trailing blocks are worth offloading."""
    return 0  # GPSIMD steals DVE's SBUF port; offload is net-negative


_CACHE = {}


def _patch_tile_tail_drain():
    """The tail drain TileContext emits carries >1 sem wait; this walrus
    build only accepts one sync wait per TPB_CTRL drain.  Split them."""
    import concourse.mybir as mybir
    import concourse.tile as tile_mod
    from concourse.tile import TileContext

    if getattr(TileContext, "_drain_patched", False):
        return
    _ScopedClock = tile_mod.ScopedClock

    def _patched(self, tick_clock, wait_clock):
        drain_inst = self.nc.sync.drain()
        wait_clock.add_sem_waits(
            drain_inst.ins, _ScopedClock({None: tick_clock.global_clock})
        )
        si = drain_inst.ins.sync_info
        if si is not None and len(si.on_wait) > 1:
            waits = list(si.on_wait)
            drain_inst.ins.sync_info = mybir.SyncInfo(
                on_wait=waits[:1], on_update=list(si.on_update)
            )
            for i in range(1, len(waits)):
                d2 = self.nc.sync.drain()
                d2.ins.sync_info = mybir.SyncInfo(on_wait=[waits[i]], on_update=[])
        self.nc.all_engine_barrier()
        assert self.sems is not None
        popped = self.nc._tile_sem_poison_stack.pop()
        assert popped is self._sem_poison
        self.nc.clear_and_free_semaphores(list(self.sems.allocated().values()))
        self.nc.all_engine_barrier()

    TileContext._drain_and_barrier = _patched
    TileContext._drain_patched = True


def _split_multi_waits(nc):
    """This walrus build accepts at most one sync-wait command per TPB
    instruction.  Move surplus waits onto same-engine NOPs inserted right
    before the owning instruction."""
    import concourse.mybir as mybir

    count = 0
    for blk in nc.m.functions[0].blocks:
        insts = list(blk.instructions)
        out = []
        changed = False
        for inst in insts:
            si = inst.sync_info
            if si is not None and len(si.on_wait) > 1:
                waits = list(si.on_wait)
                for w in waits[:-1]:
                    count += 1
                    nop = mybir.InstNoOp(
                        name=f"Wsplit-{count}", engine=inst.engine
                    )
                    nop.sync_info = mybir.SyncInfo(on_wait=[w], on_update=[])
                    out.append(nop)
                inst.sync_info = mybir.SyncInfo(
                    on_wait=[waits[-1]], on_update=list(si.on_update)
                )
                changed = True
            out.append(inst)
        if changed:
            blk.instructions = out
    return count


def _build_bass(include_bias):
    import concourse.bass as bass
    import concourse.mybir as mybir
    from concourse.masks import make_identity
    from concourse.tile import TileContext

    _patch_tile_tail_drain()

    f32 = mybir.dt.float32
    u32 = mybir.dt.uint32
    Alu = mybir.AluOpType
    Act = mybir.ActivationFunctionType

    nc = bass.Bass()
    xc = nc.dram_tensor("xc", [BC, O], f32, kind="ExternalInput")
    w1 = nc.dram_tensor("w1", [O, H], f32, kind="ExternalInput")
    w2 = nc.dram_tensor("w2", [H, H], f32, kind="ExternalInput")
    bias1 = nc.dram_tensor("bias1", [H, 1], f32, kind="ExternalInput")
    bias2 = nc.dram_tensor("bias2", [H, 1], f32, kind="ExternalInput")
    caug = nc.dram_tensor("caug", [H + 1, E * E], f32, kind="ExternalInput")
    out = nc.dram_tensor("out", [BC // 128, 128], f32, kind="ExternalOutput")

    with TileContext(nc) as tc:
        with (
            tc.tile_pool(name="consts", bufs=1) as consts,
            tc.tile_pool(name="mlp", bufs=2) as mlp,
            tc.tile_pool(name="apool", bufs=2) as apool,
            tc.tile_pool(name="work", bufs=1) as work,
            tc.tile_pool(name="ps_t", bufs=2, space="PSUM") as ps_t,
            tc.tile_pool(name="ps_m", bufs=2, space="PSUM") as ps_m,
        ):
            ident = consts.tile([128, 128], f32)
            make_identity(nc, ident)
            w1t = consts.tile([O, H], f32)
            nc.sync.dma_start(w1t, w1[:, :])
            w2t = consts.tile([H, H], f32)
            nc.sync.dma_start(w2t, w2[:, :])
            b1t = consts.tile([H, 1], f32)
            nc.sync.dma_start(b1t, bias1[:, :])
            b2t = consts.tile([H, 1], f32)
            nc.sync.dma_start(b2t, bias2[:, :])
            cgt = consts.tile([H, E * E], f32)
            nc.sync.dma_start(cgt, caug[0:H, :])
            if include_bias:
                b3r = consts.tile([1, E * E], f32)
                nc.sync.dma_start(b3r, caug[H : H + 1, :])
                onesr = consts.tile([1, 128], f32)
                nc.vector.memset(onesr, 1.0)

            detall = consts.tile([128, BC // 128], f32)

            # persistent LU scratch (sized for the largest chunk)
            NTX = max(CHUNKS)
            rcp = work.tile([128, NTX], f32)
            pv2 = work.tile([128, NTX], f32)
            nsq = work.tile([128, NTX, 2], f32)
            maskU = work.tile([128, NTX], u32)
            rowp = work.tile([128, NTX, E], f32)
            trow = work.tile([128, NTX, E], f32)
            tmp = work.tile([128, GRP, E - 1, E - 1], f32)

            toff = 0
            for c, nt in enumerate(CHUNKS):
                # ---- MLP in blocks of <= 8 tiles ----
                A = apool.tile([128, nt, E * E], f32, tag="A")
                for b0 in range(0, nt, 8):
                    bt = min(8, nt - b0)
                    bw = bt * 128
                    w0 = (toff + b0) * 128
                    xx = mlp.tile([128, bt, O], f32, tag="xx")
                    nc.sync.dma_start(
                        xx,
                        xc[w0 : w0 + bw, :].rearrange("(t p) o -> p t o", p=128),
                    )
                    xT = mlp.tile([O, bt, 128], f32, tag="xT")
                    for t in range(bt):
                        pst = ps_t.tile([128, 128], f32, tag="pst")
                        nc.tensor.transpose(pst, xx[:, t, :], ident)
                        nc.scalar.copy(xT[:, t, :], pst)

                    xTf = xT.rearrange("p t w -> p (t w)")
                    h1 = mlp.tile([H, bw], f32, tag="h1")
                    for s0 in range(0, bw, 512):
                        sl = min(512, bw - s0)
                        ph = ps_t.tile([H, 512], f32, tag="ph")
                        nc.tensor.matmul(ph[:, :sl], w1t, xTf[:, s0 : s0 + sl])
                        nc.scalar.activation(
                            h1[:, s0 : s0 + sl], ph[:, :sl], Act.Tanh, bias=b1t
                        )
                    h2a = mlp.tile([H, bw], f32, tag="h2a")
                    for s0 in range(0, bw, 512):
                        sl = min(512, bw - s0)
                        ph2 = ps_t.tile([H, 512], f32, tag="ph")
                        nc.tensor.matmul(ph2[:, :sl], w2t, h1[:, s0 : s0 + sl])
                        nc.scalar.activation(
                            h2a[0:H, s0 : s0 + sl], ph2[:, :sl], Act.Tanh, bias=b2t
                        )
                    for t in range(bt):
                        pm = ps_m.tile([128, E * E], f32, tag="pm")
                        for s in range(2):
                            nc.tensor.matmul(
                                pm[:, s * 512 : (s + 1) * 512],
                                h2a[:, t * 128 : (t + 1) * 128],
                                cgt[:, s * 512 : (s + 1) * 512],
                                start=True,
                                stop=not include_bias,
                            )
                            if include_bias:
                                nc.tensor.matmul(
                                    pm[:, s * 512 : (s + 1) * 512],
                                    onesr,
                                    b3r[:, s * 512 : (s + 1) * 512],
                                    start=False,
                                    stop=True,
                                )
                        nc.scalar.activation(A[:, b0 + t, :], pm, Act.Tanh)

                # ---- batched LU (no transpose; walkers on partitions) ----
                # Swaps negate the displaced row, so det needs no sign
                # bookkeeping; the diagonal is never touched after its step,
                # so det = product of the final diagonal.
                A4 = A.rearrange("p t (i j) -> p t i j", i=E)
                for k in range(E):
                    if NEIGHBOR_PIVOT and k < NEIGH_UNTIL and k < E - 1:
                        L = E - k
                        pcand = A[:, :, k * 33 : k * 33 + 33 : 32]
                        nc.vector.tensor_mul(nsq[:, :nt], pcand, pcand)
                        nc.vector.tensor_tensor(
                            maskU[:, :nt], nsq[:, :nt, 1], nsq[:, :nt, 0], Alu.is_gt
                        )
                        mb = maskU[:, :nt, None].broadcast_to([128, nt, L])
                        rK = A4[:, :, k, k:]
                        rK1 = A4[:, :, k + 1, k:]
                        nc.vector.tensor_scalar_mul(trow[:, :nt, :L], rK, -1.0)
                        nc.vector.copy_predicated(rK, mb, rK1)
                        nc.vector.copy_predicated(rK1, mb, trow[:, :nt, :L])

                    if k < E - 1:
                        piv = A4[:, :, k, k]
                        # r = piv / max(piv^2, clamp^2): 1/piv with a smooth,
                        # sign-correct guard near zero
                        nc.vector.tensor_mul(pv2[:, :nt], piv, piv)
                        nc.vector.tensor_scalar(
                            pv2[:, :nt], pv2[:, :nt], PIV_CLAMP * PIV_CLAMP,
                            None, Alu.max,
                        )
                        nc.vector.reciprocal(pv2[:, :nt], pv2[:, :nt])
                        nc.vector.tensor_mul(rcp[:, :nt], piv, pv2[:, :nt])
                        n = E - 1 - k
                        row = A4[:, :, k, k + 1 :]
                        nc.vector.tensor_mul(
                            rowp[:, :nt, :n],
                            row,
                            rcp[:, :nt, None].broadcast_to([128, nt, n]),
                        )
                        ngp = _gp_tiles(k, nt)
                        for g0 in range(0, nt, GRP):
                            gn = min(GRP, nt - g0)
                            col = A4[:, g0 : g0 + gn, k + 1 :, k]
                            nc.vector.tensor_mul(
                                tmp[:, :gn, :n, :n],
                                col[:, :, :, None].broadcast_to([128, gn, n, n]),
                                rowp[:, g0 : g0 + gn, None, :n].broadcast_to(
                                    [128, gn, n, n]
                                ),
                            )
                            # GPSIMD owns the leading ngp tiles so its sub
                            # starts right after the first outer group
                            for eng, t0, t1 in (
                                (nc.gpsimd, g0, min(g0 + gn, ngp)),
                                (nc.vector, max(g0, ngp), g0 + gn),
                            ):
                                if t0 >= t1:
                                    continue
                                eng.tensor_sub(
                                    A4[:, t0:t1, k + 1 :, k + 1 :],
                                    A4[:, t0:t1, k + 1 :, k + 1 :],
                                    tmp[:, t0 - g0 : t1 - g0, :n, :n],
                                )

                # det = product over the diagonal (tree reduce)
                diag = A[:, :, ::33]
                nc.vector.tensor_mul(
                    rowp[:, :nt, :16], diag[:, :, :16], diag[:, :, 16:]
                )
                nc.vector.tensor_mul(
                    rowp[:, :nt, :8], rowp[:, :nt, :8], rowp[:, :nt, 8:16]
                )
                nc.vector.tensor_mul(
                    rowp[:, :nt, :4], rowp[:, :nt, :4], rowp[:, :nt, 4:8]
                )
                nc.vector.tensor_mul(
                    rowp[:, :nt, :2], rowp[:, :nt, :2], rowp[:, :nt, 2:4]
                )
                nc.vector.tensor_mul(
                    detall[:, toff : toff + nt],
                    rowp[:, :nt, 0],
                    rowp[:, :nt, 1],
                )
                toff += nt

            # ---- emit dets: [128, 32] -> [32, 128] -> DRAM ----
            psd = ps_t.tile([BC // 128, 128], f32, tag="ph")
            nc.tensor.transpose(psd, detall, ident)
            dsb = consts.tile([BC // 128, 128], f32)
            nc.scalar.copy(dsb, psd)
            nc.sync.dma_start(out[:, :], dsb)

    nsplit = _split_multi_waits(nc)
    if nsplit:
        print(f"[kernel] split {nsplit} surplus sync waits onto NOPs")
    return nc


def _get_nc(include_bias=False):
    key = ("nc", bool(include_bias))
    if key not in _CACHE:
        _CACHE[key] = _build_bass(include_bias)
    return _CACHE[key]


def _first_nonzero_cols(x: np.ndarray) -> np.ndarray:
    """First E column indices of nonzeros of (x == 1) in row-major order."""
    cols = []
    for r in range(x.shape[0]):
        nz = np.flatnonzero(x[r] == 1)
        take = min(E - len(cols), nz.size)
        if take:
            cols.extend(nz[:take].tolist())
        if len(cols) >= E:
            break
    cols = cols[:E] + [0] * (E - len(cols))  # jnp.nonzero(size=E) zero-fill
    return np.asarray(cols, dtype=np.int64)


def kernel(x, W1, b1, W2, b2, W3, b3):
    from concourse import bass_utils

    x = np.ascontiguousarray(np.asarray(x, dtype=np.float32))
    W1 = np.asarray(W1, dtype=np.float32)
    b1 = np.asarray(b1, dtype=np.float32)
    W2 = np.asarray(W2, dtype=np.float32)
    b2 = np.asarray(b2, dtype=np.float32)
    W3 = np.asarray(W3, dtype=np.float32)
    b3 = np.asarray(b3, dtype=np.float32)

    cols = _first_nonzero_cols(x)
    csel = W3[:, cols, :].reshape(H, E * E)
    bsel = b3[cols, :].reshape(1, E * E)
    caug = np.ascontiguousarray(np.concatenate([csel, bsel], axis=0))

    shared = {
        "w1": W1,
        "w2": W2,
        "bias1": b1.reshape(H, 1),
        "bias2": b2.reshape(H, 1),
        "caug": caug,
    }
    in_maps = [
        {"xc": x[c * BC : (c + 1) * BC], **shared} for c in range(NCORES)
    ]

    nc = _get_nc(include_bias=bool(np.any(bsel)))
    res = bass_utils.run_bass_kernel_spmd(nc, in_maps, core_ids=list(range(NCORES)))
    det = np.concatenate(
        [np.asarray(res.results[c]["out"]).reshape(BC) for c in range(NCORES)]
    )
    return det.astype(np.float32)

